# revision 1
# baseline (speedup 1.0000x reference)
"""Trainium2 Bass kernel: temporal-masked MHA + top2-gated MoE layer (8 NeuronCores).

Strategy (v2):
  - data-parallel attention over B (8 batches -> 8 cores), transposed layout,
    f16 matmul inputs (psum f32), block-causal skipping (time is sorted along
    L, so the temporal mask is block-causal; the diagonal blocks still use the
    real time comparison)
  - gate logits ride a separate tiny AllToAll so top-2 routing overlaps the
    main f16 y AllToAll
  - expert FFN: w1 weight-stationary -> hT, w2 activation-stationary (hts as
    lhsT) producing eo rows directly (no output transposes); big weight DMAs
    spread across engines; double-buffered psum
"""

import math
from contextlib import ExitStack

import numpy as np

import concourse.bass as bass
import concourse.bacc as bacc
import concourse.mybir as mybir
import concourse.tile as tile
from concourse.bass_utils import run_bass_kernel_spmd
from concourse.masks import make_identity

F32 = mybir.dt.float32
F32R = mybir.dt.float32r
F16 = mybir.dt.float16
I32 = mybir.dt.int32
AX = mybir.AxisListType
OP = mybir.AluOpType
ACT = mybir.ActivationFunctionType
P = 128

FULL = dict(L=512, B=8, E=1024, H=16, HID=4096, NE=5, NC=8)


def make_cfg(d):
    c = dict(d)
    c["CAP"] = max(min(c["B"], int(c["B"] * 2.0 / c["NE"])), 4)
    c["D"] = c["E"] // c["H"]
    assert c["D"] == 64, "head dim assumed 64"
    assert c["B"] == c["NC"]
    c["LC"] = c["L"] // c["NC"]          # L-groups per core
    c["TOK"] = c["LC"] * c["B"]          # MoE tokens per core
    assert c["TOK"] % P == 0
    assert c["L"] % P == 0
    assert c["NE"] <= 8
    c["GCAP"] = c["LC"] * c["CAP"]       # slots per expert per core
    return c


def tl(pool, shape, dtype=F32, *, tag, bufs=None):
    return pool.tile(list(shape), dtype, tag=tag, name=tag, bufs=bufs)


def build_bass(c):
    nc = bacc.Bacc("TRN2", target_bir_lowering=False, debug=False,
                   num_devices=c["NC"])
    L, B, E, H, HID, NE = c["L"], c["B"], c["E"], c["H"], c["HID"], c["NE"]
    CAP, LC, TOK, GCAP = c["CAP"], c["LC"], c["TOK"], c["GCAP"]
    KT = E // P                       # k-tiles over E
    MTOK = L // P                     # token tiles (attention, per batch)
    NTOKT = TOK // P                  # token tiles (MoE)
    NHT = 2 * E // P                  # qk row tiles
    HIDT = HID // P
    GPT = P // B                      # groups per 128-token tile
    spt = min(P, GCAP)                # slots per slot-tile
    nslt = (GCAP + P - 1) // P        # slot tiles per expert
    tpst = spt // (GPT * CAP)         # token-tiles per slot-tile
    sc = 1.0 / math.sqrt(64)

    # ---- I/O ----
    dt_ = nc.dram_tensor
    xT = dt_("xT", [E, L], F32, kind="ExternalInput")[:]
    tcol = dt_("tcol", [L, 1], F32, kind="ExternalInput")[:]
    trep = dt_("trep", [P, L], F32, kind="ExternalInput")[:]
    wqkvT = dt_("wqkvT", [E, 3 * E], F16, kind="ExternalInput")[:]
    bqk = dt_("bqk", [2 * E, 1], F32, kind="ExternalInput")[:]
    bvrep = dt_("bvrep", [P, E], F32, kind="ExternalInput")[:]
    woutT = dt_("woutT", [E, E], F16, kind="ExternalInput")[:]
    bout = dt_("bout", [E, 1], F32, kind="ExternalInput")[:]
    ln1g = dt_("ln1g", [E, 1], F32, kind="ExternalInput")[:]
    ln1b = dt_("ln1b", [E, 1], F32, kind="ExternalInput")[:]
    ln2g = dt_("ln2grep", [P, E], F32, kind="ExternalInput")[:]
    ln2b = dt_("ln2brep", [P, E], F32, kind="ExternalInput")[:]
    gw = dt_("gatew", [E, NE], F32, kind="ExternalInput")[:]   # pre-scaled by ln1_g
    gc0 = dt_("gatec0", [NE, 1], F32, kind="ExternalInput")[:]  # b @ gw
    gc1v = dt_("gatec1", [NE, 1], F32, kind="ExternalInput")[:]  # g @ gw
    w1 = dt_("w1", [NE, E, HID], F16, kind="ExternalInput")[:]
    w2 = dt_("w2", [NE, HID, E], F16, kind="ExternalInput")[:]
    out = dt_("out", [TOK, E], F32, kind="ExternalOutput")[:]

    # ---- host-side constant tables (baked into the NEFF) ----
    tri = np.zeros((P, P), np.float32)       # strict-lower within B-groups
    ob = np.zeros((P, P), np.float32)        # all-ones within B-groups
    for i in range(P):
        for j in range(P):
            if i // B == j // B:
                ob[i, j] = 1.0
                if i < j:
                    tri[i, j] = 1.0
    nsel = np.zeros((P, GPT), np.float32)
    for i in range(P):
        nsel[i, i // B] = float(i % B)
    iotac4 = np.tile(np.arange(CAP, dtype=np.float32), (P, NTOKT))
    iotae4 = np.tile(np.arange(NE, dtype=np.float32), (P, NTOKT))
    gbase10 = np.zeros((spt, NE * nslt), np.float32)
    for e_ in range(NE):
        for st in range(nslt):
            for p in range(spt):
                gbase10[p, e_ * nslt + st] = float(B * ((st * P + p) // CAP))
    gb2c = np.zeros((P, NTOKT), np.float32)
    for t in range(NTOKT):
        for p in range(P):
            gb2c[p, t] = float(CAP * ((t * P + p) // B))
    egcap = np.tile(np.repeat(np.arange(NE, dtype=np.float32) * GCAP, 1),
                    (P, NTOKT))  # [P, NTOKT*NE]: e*GCAP per (t,e) column
    # one-hot head-selector columns for the batched softmax denominator
    ehall = np.zeros((P, H * H), np.float16)
    for h in range(H):
        ehall[:, h * H + h] = 1.0
    # row-h selector/replicator: rep_h = sel[h].T @ recD  (row h -> 64 rows)
    selall = np.zeros((H, H * 64), np.float16)
    for h in range(H):
        selall[h, h * 64:(h + 1) * 64] = 1.0

    # round-robin engines for weight-stream DMA triggering (vector cannot
    # trigger DMAs; gpsimd's collectives/gathers all precede these in
    # program order so its queue has slack during the FFN)
    dma_engines = [nc.sync, nc.scalar, nc.gpsimd]
    _ecnt = [0]

    def wdma(dst, src):
        e = dma_engines[_ecnt[0] % len(dma_engines)]
        _ecnt[0] += 1
        e.dma_start(dst, src)

    with tile.TileContext(nc) as tc, ExitStack() as ctx:
        cst = ctx.enter_context(tc.tile_pool(name="cst", bufs=1))
        dram = ctx.enter_context(tc.tile_pool(name="dram", bufs=1, space="DRAM"))
        pB = ctx.enter_context(tc.tile_pool(name="pB", bufs=1))
        # pB is the bottom of the SBUF stack and must not grow after later
        # pools stack above it -- allocate every persistent tile up front
        ln2g_sb = tl(pB, [P, E], F32, tag="ln2g")
        ln2b_sb = tl(pB, [P, E], F32, tag="ln2b")
        acc = [tl(pB, [P, E], F32, tag=f"acc{tt}") for tt in range(NTOKT)]
        gsel = tl(pB, [P, NTOKT * NE], F32, tag="gsel")
        gca = tl(pB, [P, NTOKT], F32, tag="gca")
        gcb = tl(pB, [P, NTOKT], F32, tag="gcb")
        idxsel = {}
        for e_ in range(NE):
            for tt in range(NTOKT):
                idxsel[(e_, tt)] = tl(pB, [P, 1], I32, tag=f"ix{e_}_{tt}")
        islot_i = tl(pB, [spt, NE * nslt], I32, tag="islot_i")

        def const_tile(arr, tag):
            ap = nc.inline_tensor(np.ascontiguousarray(arr), name=tag)[:]
            t = tl(cst, list(arr.shape), F32, tag=tag)
            nc.gpsimd.dma_start(t[:], ap)
            return t

        ident = tl(cst, [P, P], F32, tag="ident")
        make_identity(nc, ident[:])
        ident16 = tl(cst, [P, P], F16, tag="ident16")
        make_identity(nc, ident16[:])
        ones_t = tl(cst, [P, 1], F32, tag="ones")
        nc.vector.memset(ones_t[:], 1.0)
        onesr_t = tl(cst, [1, P], F32, tag="onesr")
        nc.vector.memset(onesr_t[:], 1.0)
        onesr16 = tl(cst, [1, P], F16, tag="onesr16")
        nc.vector.memset(onesr16[:], 1.0)
        ones16 = tl(cst, [P, 1], F16, tag="ones16")
        nc.vector.memset(ones16[:], 1.0)
        tri_t = const_tile(tri, "tri")
        ob_t = const_tile(ob, "ob")
        nsel_t = const_tile(nsel, "nsel")
        iotac_t = const_tile(iotac4, "iotac4")
        iotae_t = const_tile(iotae4, "iotae4")
        gb2_t = const_tile(gb2c, "gb2c")
        egcap_t = const_tile(egcap, "egcap")
        gbase10_t = const_tile(gbase10, "gbase10")
        ehbig = nc.inline_tensor(ehall, name="ehall")[:]
        ehb_t = tl(cst, [P, H * H], F16, tag="ehbig")
        nc.gpsimd.dma_start(ehb_t[:], ehbig)
        eh_ts = [ehb_t[:, h * H:(h + 1) * H] for h in range(H)]
        selbig = nc.inline_tensor(selall, name="selall")[:]
        selb_t = tl(cst, [H, H * 64], F16, tag="selbig")
        nc.gpsimd.dma_start(selb_t[:], selbig)
        sel_ts = [selb_t[:, h * 64:(h + 1) * 64] for h in range(H)]
        gc0_t = tl(cst, [NE, 1], F32, tag="gc0")
        nc.gpsimd.dma_start(gc0_t[:], gc0)
        gc1_t = tl(cst, [NE, 1], F32, tag="gc1")
        nc.gpsimd.dma_start(gc1_t[:], gc1v)

        ROWW = E + 16   # y (f16) + gate logits packed as f16 hi/lo pairs
        send16 = tl(dram, [L, ROWW], F16, tag="send16")

        # =========================================================
        # PHASE A: attention for this core's batch (transposed layout, f16)
        # =========================================================
        y16 = []
        with tc.tile_pool(name="pA", bufs=1) as pA:
            # x loads lead the sync queue; small operands ride scalar so
            # the const stream on gpsimd never gates the critical path
            xt = []
            xt16 = []
            for k in range(KT):
                t = tl(pA, [P, L], F32, tag=f"xt{k}")
                nc.sync.dma_start(t[:], xT[k * P:(k + 1) * P, :])
                xt.append(t)
                t16 = tl(pA, [P, L], F16, tag=f"xt16_{k}")
                nc.vector.tensor_copy(t16[:], t[:])
                xt16.append(t16)
            bqk_t = tl(pA, [P, NHT], F32, tag="bqk")
            nc.scalar.dma_start(bqk_t[:], bqk.rearrange("(m p) o -> p (m o)", p=P))
            bv_t = []
            for nn in range(2):
                t = tl(pA, [P, 512], F32, tag=f"bv{nn}")
                nc.scalar.dma_start(t[:], bvrep[:, nn * 512:(nn + 1) * 512])
                bv_t.append(t)
            tcol_t = tl(pA, [P, MTOK], F32, tag="tcol")
            nc.scalar.dma_start(tcol_t[:], tcol.rearrange("(m p) o -> p (m o)", p=P))
            trep_t = tl(pA, [P, L], F32, tag="trep")
            nc.scalar.dma_start(trep_t[:], trep)
            gw_t = tl(pA, [P, KT * NE], F32, tag="gw")
            nc.scalar.dma_start(gw_t[:].rearrange("p (k e) -> p k e", e=NE),
                                gw.rearrange("(k p) e -> p k e", p=P))
            bout_t = tl(pA, [P, KT], F32, tag="bout")
            nc.scalar.dma_start(bout_t[:], bout.rearrange("(m p) o -> p (m o)", p=P))
            ln1g_t = tl(pA, [P, KT], F32, tag="ln1g")
            nc.scalar.dma_start(ln1g_t[:], ln1g.rearrange("(m p) o -> p (m o)", p=P))
            ln1b_t = tl(pA, [P, KT], F32, tag="ln1b")
            nc.scalar.dma_start(ln1b_t[:], ln1b.rearrange("(m p) o -> p (m o)", p=P))

            # ---- qkT = wqkv[:2E] @ x^T (f16); V token-major w/ ones col
            qk = [tl(pA, [P, L], F16, tag=f"qk{m}") for m in range(NHT)]
            vt = [tl(pA, [P, E], F16, tag=f"vt{m}") for m in range(MTOK)]
            with tc.tile_pool(name="wp", bufs=2) as wp, \
                 tc.tile_pool(name="psQ", bufs=2, space="PSUM") as psQ:
                for mg in range(NHT // 4):
                    wq = []
                    for k in range(KT):
                        t = tl(wp, [P, 512], F16, tag=f"wq{k}")
                        nc.sync.dma_start(
                            t[:], wqkvT[k * P:(k + 1) * P,
                                        mg * 512:(mg + 1) * 512])
                        wq.append(t)
                    for j in range(4):
                        m = mg * 4 + j
                        ps = tl(psQ, [P, L], F32, tag="ps")
                        for k in range(KT):
                            nc.tensor.matmul(
                                ps[:], lhsT=wq[k][:, j * P:(j + 1) * P],
                                rhs=xt16[k][:], start=(k == 0),
                                stop=(k == KT - 1))
                        nc.vector.tensor_scalar_add(qk[m][:], ps[:],
                                                    bqk_t[:, m:m + 1])
                # V: x-stationary, weight cols streamed
                wv = {}
                for nn in range(2):
                    for k in range(KT):
                        t = tl(wp, [P, 512], F16, tag=f"wv{nn}_{k}", bufs=1)
                        nc.sync.dma_start(
                            t[:], wqkvT[k * P:(k + 1) * P,
                                        2 * E + nn * 512:2 * E + (nn + 1) * 512])
                        wv[(nn, k)] = t
                for mt in range(MTOK):
                    for nn in range(2):
                        ps = tl(psQ, [P, 512], F32, tag="ps")
                        for k in range(KT):
                            nc.tensor.matmul(
                                ps[:], lhsT=xt16[k][:, mt * P:(mt + 1) * P],
                                rhs=wv[(nn, k)][:], start=(k == 0),
                                stop=(k == KT - 1))
                        nc.vector.tensor_add(
                            vt[mt][:, nn * 512:(nn + 1) * 512], ps[:],
                            bv_t[nn][:])

            # ---- diagonal temporal masks (0 / -8e9; exp scale 1/8 -> -1e9)
            maskd = [tl(pA, [P, P], F32, tag=f"md{kt}") for kt in range(MTOK)]
            for kt in range(MTOK):
                nc.vector.tensor_tensor(
                    out=maskd[kt][:],
                    in0=tcol_t[:, kt:kt + 1].to_broadcast([P, P]),
                    in1=trep_t[:, kt * P:(kt + 1) * P], op=OP.is_gt)
                nc.vector.tensor_scalar_mul(maskd[kt][:], maskd[kt][:],
                                            -1e9 / sc)

            # ---- heads (block-causal: key tile kt only attends q >= kt*P)
            # denominators for ALL heads accumulate into one [H, L] psum via
            # one-hot lhsT columns; a single batched reciprocal replaces 16
            # serial [1,L] reciprocals (3.3us each)
            attnT = [tl(pA, [P, L], F16, tag=f"at{k}") for k in range(KT)]
            avS = [tl(pA, [64, L], F16, tag=f"avS{h}") for h in range(H)]
            with tc.tile_pool(name="pp", bufs=2) as pp, \
                 tc.tile_pool(name="smp", bufs=3) as smp, \
                 tc.tile_pool(name="psS", bufs=1, space="PSUM") as psS, \
                 tc.tile_pool(name="psD", bufs=1, space="PSUM") as psDp, \
                 tc.tile_pool(name="psAV", bufs=2, space="PSUM") as psAV:
                psD = tl(psDp, [H, L], F32, tag="psD")
                for h in range(H):
                    mq, rq = (h * 64) // P, (h * 64) % P
                    mk, rk = (E + h * 64) // P, (E + h * 64) % P
                    pts = []
                    for kt in range(MTOK):
                        N = L - kt * P
                        sps = tl(psS, [P, N], F32, tag=f"sps{kt}")
                        nc.tensor.matmul(
                            sps[:],
                            lhsT=qk[mk][rk:rk + 64, kt * P:(kt + 1) * P],
                            rhs=qk[mq][rq:rq + 64, kt * P:L],
                            start=True, stop=True)
                        nc.vector.tensor_add(sps[:, 0:P], sps[:, 0:P],
                                             maskd[kt][:])
                        pt_ = tl(pp, [P, N], F16, tag=f"pt{kt}")
                        nc.scalar.activation(pt_[:], sps[:], ACT.Exp, scale=sc)
                        pts.append(pt_)
                        nc.tensor.matmul(
                            psD[:, kt * P:L], lhsT=eh_ts[h], rhs=pt_[:],
                            start=(h == 0 and kt == 0),
                            stop=(h == H - 1 and kt == MTOK - 1),
                            skip_group_check=True)
                    av = tl(psAV, [64, L], F32, tag="av")
                    for qt in range(MTOK):
                        for kt in range(qt + 1):
                            nc.tensor.matmul(
                                av[:, qt * P:(qt + 1) * P],
                                lhsT=vt[kt][:, h * 64:h * 64 + 64],
                                rhs=pts[kt][:, (qt - kt) * P:(qt - kt + 1) * P],
                                start=(kt == 0), stop=(kt == qt))
                    nc.vector.tensor_copy(avS[h][:], av[:])
                # batched normalization
                recD = tl(smp, [H, L], F32, tag="recD")
                nc.vector.reciprocal(recD[:], psD[:])
                recD16 = tl(smp, [H, L], F16, tag="recD16")
                nc.vector.tensor_copy(recD16[:], recD[:])
                for h in range(H):
                    mq, rq = (h * 64) // P, (h * 64) % P
                    rep_ps = tl(psAV, [64, L], F32, tag="repps", bufs=1)
                    nc.tensor.matmul(rep_ps[:], lhsT=sel_ts[h],
                                     rhs=recD16[:], start=True,
                                     stop=True)
                    rep = tl(smp, [64, L], F16, tag="rep")
                    nc.vector.tensor_copy(rep[:], rep_ps[:])
                    nc.vector.tensor_mul(attnT[mq][rq:rq + 64, :], avS[h][:],
                                         rep[:])

            # ---- out-proj + residual (into xt -> zT)
            with tc.tile_pool(name="wp2", bufs=2) as wp2, \
                 tc.tile_pool(name="psO", bufs=2, space="PSUM") as psO:
                for mg in range(KT // 4):
                    wo = []
                    for k in range(KT):
                        t = tl(wp2, [P, 512], F16, tag=f"wo{k}")
                        nc.sync.dma_start(
                            t[:], woutT[k * P:(k + 1) * P,
                                        mg * 512:(mg + 1) * 512])
                        wo.append(t)
                    for j in range(4):
                        m = mg * 4 + j
                        ps = tl(psO, [P, L], F32, tag="ps")
                        for k in range(KT):
                            nc.tensor.matmul(ps[:],
                                             lhsT=wo[k][:, j * P:(j + 1) * P],
                                             rhs=attnT[k][:],
                                             start=(k == 0), stop=(k == KT - 1))
                        nc.vector.tensor_scalar_add(ps[:], ps[:],
                                                    bout_t[:, m:m + 1])
                        nc.vector.tensor_add(xt[m][:], ps[:], xt[m][:])  # zT

            # ---- LN1 stats + pre-LN gate logits
            # logits = rstd*(z @ (g.*gw)) - (rstd*mu)*(g@gw) + b@gw, so the
            # gate matmul runs on z directly and the logits A2A departs
            # before the y normalization loop
            for k in range(KT):
                y16.append(tl(pA, [P, L], F16, tag=f"y16_{k}"))
            with tc.tile_pool(name="lnp", bufs=3) as lnp, \
                 tc.tile_pool(name="gp", bufs=2) as gp, \
                 tc.tile_pool(name="psG", bufs=2, space="PSUM") as psG, \
                 tc.tile_pool(name="psL", bufs=1, space="PSUM") as psL:
                mu_ps = tl(psL, [1, L], F32, tag="mu")
                sq_ps = tl(psL, [1, L], F32, tag="sq")
                for k in range(KT):
                    z16 = tl(lnp, [P, L], F16, tag="z16")
                    nc.vector.tensor_copy(z16[:], xt[k][:])
                    nc.tensor.matmul(mu_ps[:], lhsT=ones16[:], rhs=z16[:],
                                     start=(k == 0), stop=(k == KT - 1))
                    sqt = tl(lnp, [P, L], F16, tag="sqt")
                    nc.scalar.activation(sqt[:], xt[k][:], ACT.Square)
                    nc.tensor.matmul(sq_ps[:], lhsT=ones16[:], rhs=sqt[:],
                                     start=(k == 0), stop=(k == KT - 1))
                gps = tl(psG, [NE, L], F32, tag="gps", bufs=1)
                for k in range(KT):
                    nc.tensor.matmul(
                        gps[:], lhsT=gw_t[:, k * NE:(k + 1) * NE],
                        rhs=xt[k][:], start=(k == 0), stop=(k == KT - 1))
                mu_r = tl(lnp, [1, L], F32, tag="mu_r")
                nc.vector.tensor_scalar_mul(mu_r[:], mu_ps[:], 1.0 / E)
                var_r = tl(lnp, [1, L], F32, tag="var_r")
                nc.vector.tensor_scalar_mul(var_r[:], sq_ps[:], 1.0 / E)
                mu2 = tl(lnp, [1, L], F32, tag="mu2")
                nc.vector.tensor_mul(mu2[:], mu_r[:], mu_r[:])
                nc.vector.tensor_sub(var_r[:], var_r[:], mu2[:])
                nc.vector.tensor_scalar_add(var_r[:], var_r[:], 1e-5)
                nc.scalar.sqrt(var_r[:], var_r[:])
                rstd_r = tl(lnp, [1, L], F32, tag="rstd_r")
                nc.vector.reciprocal(rstd_r[:], var_r[:])
                murst = tl(lnp, [1, L], F32, tag="murst")
                nc.vector.tensor_mul(murst[:], mu_r[:], rstd_r[:])
                mur16 = tl(lnp, [1, L], F16, tag="mur16")
                nc.vector.tensor_copy(mur16[:], mu_r[:])
                rstdr16 = tl(lnp, [1, L], F16, tag="rstdr16")
                nc.vector.tensor_copy(rstdr16[:], rstd_r[:])
                murst16 = tl(lnp, [1, L], F16, tag="murst16")
                nc.vector.tensor_copy(murst16[:], murst[:])
                # gate affine correction on [NE, L] (one psum bank, reused)
                rst5_ps = tl(psG, [NE, L], F32, tag="b5", bufs=1)
                nc.tensor.matmul(rst5_ps[:], lhsT=onesr16[:, 0:NE],
                                 rhs=rstdr16[:], start=True, stop=True)
                rst5 = tl(gp, [NE, L], F32, tag="rst5sb")
                nc.vector.tensor_copy(rst5[:], rst5_ps[:])
                mrst5_ps = tl(psG, [NE, L], F32, tag="b5", bufs=1)
                nc.tensor.matmul(mrst5_ps[:], lhsT=onesr16[:, 0:NE],
                                 rhs=murst16[:], start=True, stop=True)
                mrst5 = tl(gp, [NE, L], F32, tag="mrst5sb")
                nc.vector.tensor_scalar_mul(mrst5[:], mrst5_ps[:], gc1_t[:])
                lg_sb = tl(gp, [NE, L], F32, tag="lg_sb")
                nc.vector.tensor_tensor(out=lg_sb[:], in0=gps[:], in1=rst5[:],
                                        op=OP.mult)
                nc.vector.tensor_sub(lg_sb[:], lg_sb[:], mrst5[:])
                nc.vector.tensor_scalar_add(lg_sb[:], lg_sb[:], gc0_t[:])
                # logits -> f16 hi + residual lo (reconstructs to ~1e-7)
                lghi = tl(gp, [NE, L], F16, tag="lghi")
                nc.vector.tensor_copy(lghi[:], lg_sb[:])
                lghi32 = tl(gp, [NE, L], F32, tag="lghi32")
                nc.vector.tensor_copy(lghi32[:], lghi[:])
                lglo = tl(gp, [NE, L], F16, tag="lglo")
                nc.vector.tensor_sub(lglo[:], lg_sb[:], lghi32[:])

                # ---- y16 = LN1(z) (f16 direct; one psum bank reused)
                murep_ps = tl(psL, [P, L], F32, tag="brep", bufs=1)
                nc.tensor.matmul(murep_ps[:], lhsT=onesr16[:],
                                 rhs=mur16[:], start=True, stop=True)
                mu_rep = tl(lnp, [P, L], F32, tag="mu_rep")
                nc.vector.tensor_copy(mu_rep[:], murep_ps[:])
                rsrep_ps = tl(psL, [P, L], F32, tag="brep", bufs=1)
                nc.tensor.matmul(rsrep_ps[:], lhsT=onesr16[:],
                                 rhs=rstdr16[:], start=True, stop=True)
                rstd_rep = tl(lnp, [P, L], F32, tag="rstd_rep")
                nc.vector.tensor_copy(rstd_rep[:], rsrep_ps[:])
                for k in range(KT):
                    t1 = tl(lnp, [P, L], F32, tag="t1")
                    nc.vector.tensor_sub(t1[:], xt[k][:], mu_rep[:])
                    nc.vector.tensor_mul(t1[:], t1[:], rstd_rep[:])
                    nc.vector.tensor_scalar(
                        out=y16[k][:], in0=t1[:], scalar1=ln1g_t[:, k:k + 1],
                        scalar2=ln1b_t[:, k:k + 1], op0=OP.mult, op1=OP.add)

                # ---- y + packed logits -> token-major f16 send buffer
                for ct in range(MTOK):
                    yrow = tl(gp, [P, ROWW], F16, tag="yrow")
                    for k in range(KT):
                        tpY = tl(psG, [P, P], F16, tag="tpY")
                        nc.tensor.transpose(tpY[:], y16[k][:, ct * P:(ct + 1) * P],
                                            ident16[:])
                        nc.vector.tensor_copy(yrow[:, k * P:(k + 1) * P], tpY[:])
                    tp2 = tl(psG, [P, NE], F16, tag="tp2", bufs=1)
                    nc.tensor.transpose(tp2[:, 0:NE],
                                        lghi[:, ct * P:(ct + 1) * P],
                                        ident16[0:NE, 0:NE])
                    nc.vector.tensor_copy(yrow[:, E:E + NE], tp2[:, 0:NE])
                    tp2b = tl(psG, [P, NE], F16, tag="tp2", bufs=1)
                    nc.tensor.transpose(tp2b[:, 0:NE],
                                        lglo[:, ct * P:(ct + 1) * P],
                                        ident16[0:NE, 0:NE])
                    nc.vector.tensor_copy(yrow[:, E + 8:E + 8 + NE], tp2b[:, 0:NE])
                    nc.sync.dma_start(send16[ct * P:(ct + 1) * P, :], yrow[:])

        # FFN weight pools open once attention SBUF is released (the
        # expert-0 half-0 prefetch in the FFN section streams during A2A)
        wf = ctx.enter_context(tc.tile_pool(name="wf", bufs=2))
        w2p = ctx.enter_context(tc.tile_pool(name="w2p", bufs=6))
        tl(w2p, [P, E], F16, tag="w2r")   # reserve: pool must not grow later

        # =========================================================
        # AllToAll + permute to group-major token order
        # =========================================================
        recv16 = tl(dram, [L, ROWW], F16, tag="recv16")
        nc.gpsimd.collective_compute(
            "AllToAll", OP.bypass,
            replica_groups=[list(range(c["NC"]))],
            ins=[send16[:].opt()], outs=[recv16[:].opt()])
        lgbuf = tl(dram, [TOK, 16], F16, tag="lgbuf")
        nc.sync.dma_start(
            lgbuf[:].rearrange("(l i) r -> l i r", i=c["NC"]),
            recv16[:][:, E:E + 16].rearrange("(i l) r -> l i r", i=c["NC"]))

        # =========================================================
        # PHASE B: top-2 routing with capacity (overlaps the y A2A)
        # all 4 token-tiles processed as one [P, 4*NE] batch via 3D APs
        # =========================================================
        TN = NTOKT * NE
        nmat_d = tl(dram, [NTOKT, GPT, NE * CAP], F32, tag="nmat_d")

        def r3(ap):
            return ap.rearrange("p (t e) -> p t e", e=NE)

        with tc.tile_pool(name="rt", bufs=1) as rt, \
             tc.tile_pool(name="psR", bufs=1, space="PSUM") as psR:
            lg = tl(rt, [P, TN], F32, tag="lg")
            for tt in range(NTOKT):
                lgp = tl(rt, [P, 16], F16, tag="lgp", bufs=4)
                nc.scalar.dma_start(lgp[:], lgbuf[tt * P:(tt + 1) * P, :])
                nc.vector.tensor_add(lg[:, tt * NE:(tt + 1) * NE],
                                     lgp[:, 0:NE], lgp[:, 8:8 + NE])
            # softmax over NE per tile (logits are small: no max-sub needed)
            ex = tl(rt, [P, TN], F32, tag="ex")
            nc.scalar.activation(ex[:], lg[:], ACT.Exp)
            sm = tl(rt, [P, NTOKT], F32, tag="sm")
            nc.vector.reduce_sum(sm[:], r3(ex[:]), axis=AX.X)
            rcp = tl(rt, [P, NTOKT], F32, tag="rcp")
            nc.vector.reciprocal(rcp[:], sm[:])
            raw = tl(rt, [P, TN], F32, tag="raw")
            nc.vector.tensor_tensor(
                out=r3(raw[:]), in0=r3(ex[:]),
                in1=rcp[:].unsqueeze(2).to_broadcast([P, NTOKT, NE]),
                op=OP.mult)

            def top1(rawt, tag):
                g = tl(rt, [P, NTOKT], F32, tag=f"g{tag}")
                nc.vector.reduce_max(g[:], r3(rawt), axis=AX.X)
                eq = tl(rt, [P, TN], F32, tag=f"eq{tag}")
                nc.vector.tensor_tensor(
                    out=r3(eq[:]), in0=r3(rawt),
                    in1=g[:].unsqueeze(2).to_broadcast([P, NTOKT, NE]),
                    op=OP.is_ge)
                cs = tl(rt, [P, TN], F32, tag=f"cs{tag}")
                nc.vector.memset(r3(cs[:])[:, :, 0:1], 0.0)
                for j in range(1, NE):
                    nc.vector.tensor_add(r3(cs[:])[:, :, j:j + 1],
                                         r3(cs[:])[:, :, j - 1:j],
                                         r3(eq[:])[:, :, j - 1:j])
                fst = tl(rt, [P, TN], F32, tag=f"fst{tag}")
                nc.vector.tensor_scalar(out=fst[:], in0=cs[:], scalar1=0.5,
                                        scalar2=None, op0=OP.is_lt)
                m_ = tl(rt, [P, TN], F32, tag=f"m{tag}")
                nc.vector.tensor_mul(m_[:], eq[:], fst[:])
                return g, m_

            g1, m1r = top1(raw[:], "1")
            raw2 = tl(rt, [P, TN], F32, tag="raw2")
            nc.vector.tensor_mul(raw2[:], raw[:], m1r[:])
            nc.vector.tensor_sub(raw2[:], raw[:], raw2[:])
            g2, m2r = top1(raw2[:], "2")
            den = tl(rt, [P, NTOKT], F32, tag="den")
            nc.vector.tensor_add(den[:], g1[:], g2[:])
            nc.vector.tensor_scalar_add(den[:], den[:], 1e-9)
            rd = tl(rt, [P, NTOKT], F32, tag="rd")
            nc.vector.reciprocal(rd[:], den[:])
            g1n = tl(rt, [P, NTOKT], F32, tag="g1n")
            nc.vector.tensor_mul(g1n[:], g1[:], rd[:])
            g2n = tl(rt, [P, NTOKT], F32, tag="g2n")
            nc.vector.tensor_mul(g2n[:], g2[:], rd[:])

            # capacity by position within group (cumsum over tokens = tri/ob
            # matmuls; batched over all 4 tiles)
            pos1 = tl(psR, [P, TN], F32, tag="pos1")
            nc.tensor.matmul(pos1[:], lhsT=tri_t[:], rhs=m1r[:],
                             start=True, stop=True)
            keep1 = tl(rt, [P, TN], F32, tag="keep1")
            nc.vector.tensor_scalar(out=keep1[:], in0=pos1[:],
                                    scalar1=CAP - 0.5, scalar2=None,
                                    op0=OP.is_lt)
            m1 = tl(rt, [P, TN], F32, tag="m1k")
            nc.vector.tensor_mul(m1[:], m1r[:], keep1[:])
            pos2 = tl(psR, [P, TN], F32, tag="pos2")
            nc.tensor.matmul(pos2[:], lhsT=tri_t[:], rhs=m2r[:],
                             start=True, stop=False)
            nc.tensor.matmul(pos2[:], lhsT=ob_t[:], rhs=m1[:],
                             start=False, stop=True)
            keep2 = tl(rt, [P, TN], F32, tag="keep2")
            nc.vector.tensor_scalar(out=keep2[:], in0=pos2[:],
                                    scalar1=CAP - 0.5, scalar2=None,
                                    op0=OP.is_lt)
            m2 = tl(rt, [P, TN], F32, tag="m2k")
            nc.vector.tensor_mul(m2[:], m2r[:], keep2[:])

            def dotE(a_ap, b_ap, tag):
                t5 = tl(rt, [P, TN], F32, tag=f"t5{tag}")
                nc.vector.tensor_mul(t5[:], a_ap, b_ap)
                o = tl(rt, [P, NTOKT], F32, tag=f"o{tag}")
                nc.vector.reduce_sum(o[:], r3(t5[:]), axis=AX.X)
                return o

            m1f = tl(rt, [P, NTOKT], F32, tag="m1f")
            nc.vector.reduce_sum(m1f[:], r3(m1[:]), axis=AX.X)
            m2f = tl(rt, [P, NTOKT], F32, tag="m2f")
            nc.vector.reduce_sum(m2f[:], r3(m2[:]), axis=AX.X)
            nc.vector.tensor_mul(gca[:], g1n[:], m1f[:])
            nc.vector.tensor_mul(gcb[:], g2n[:], m2f[:])
            p1 = dotE(pos1[:], m1[:], "p1")
            p2 = dotE(pos2[:], m2[:], "p2")
            e1 = dotE(iotae_t[:], m1[:], "e1")
            e2 = dotE(iotae_t[:], m2[:], "e2")
            # capacity-dropped ranks alias to expert 0 (sums of zeroed masks);
            # bump them to a sentinel so they match no expert in m1e/m2e
            sent = tl(rt, [P, NTOKT], F32, tag="sent")
            nc.vector.tensor_scalar(out=sent[:], in0=m1f[:], scalar1=-64.0,
                                    scalar2=64.0, op0=OP.mult, op1=OP.add)
            nc.vector.tensor_add(e1[:], e1[:], sent[:])
            nc.vector.tensor_scalar(out=sent[:], in0=m2f[:], scalar1=-64.0,
                                    scalar2=64.0, op0=OP.mult, op1=OP.add)
            nc.vector.tensor_add(e2[:], e2[:], sent[:])

            # per-(expert, tile) gather indices + gate weights for the
            # streaming combine: idx = e*GCAP + group*CAP + pos if the token
            # routed to e (rank 1 or 2), else the shared zero row
            ZROW = float(NE * GCAP)
            lidx1 = tl(rt, [P, NTOKT], F32, tag="lidx1")
            nc.vector.tensor_add(lidx1[:], p1[:], gb2_t[:])
            lidx2 = tl(rt, [P, NTOKT], F32, tag="lidx2")
            nc.vector.tensor_add(lidx2[:], p2[:], gb2_t[:])
            m1e = tl(rt, [P, TN], F32, tag="m1e")
            nc.vector.tensor_tensor(
                out=r3(m1e[:]), in0=e1[:].unsqueeze(2).to_broadcast([P, NTOKT, NE]),
                in1=r3(iotae_t[:]), op=OP.is_equal)
            m2e = tl(rt, [P, TN], F32, tag="m2e")
            nc.vector.tensor_tensor(
                out=r3(m2e[:]), in0=e2[:].unsqueeze(2).to_broadcast([P, NTOKT, NE]),
                in1=r3(iotae_t[:]), op=OP.is_equal)
            ga_ = tl(rt, [P, TN], F32, tag="ga_")
            nc.vector.tensor_tensor(
                out=r3(ga_[:]), in0=gca[:].unsqueeze(2).to_broadcast([P, NTOKT, NE]),
                in1=r3(m1e[:]), op=OP.mult)
            gb_ = tl(rt, [P, TN], F32, tag="gb_")
            nc.vector.tensor_tensor(
                out=r3(gb_[:]), in0=gcb[:].unsqueeze(2).to_broadcast([P, NTOKT, NE]),
                in1=r3(m2e[:]), op=OP.mult)
            nc.vector.tensor_add(gsel[:], ga_[:], gb_[:])
            ia_ = tl(rt, [P, TN], F32, tag="ia_")
            nc.vector.tensor_tensor(
                out=r3(ia_[:]), in0=lidx1[:].unsqueeze(2).to_broadcast([P, NTOKT, NE]),
                in1=r3(m1e[:]), op=OP.mult)
            ib_ = tl(rt, [P, TN], F32, tag="ib_")
            nc.vector.tensor_tensor(
                out=r3(ib_[:]), in0=lidx2[:].unsqueeze(2).to_broadcast([P, NTOKT, NE]),
                in1=r3(m2e[:]), op=OP.mult)
            # idx = (lidx1+e*G)*m1e + (lidx2+e*G)*m2e + ZROW*(1-m1e-m2e)
            # built as: (lidx1*m1e + lidx2*m2e) + e*G*(m1e+m2e) + ZROW*(1-..)
            zm = tl(rt, [P, TN], F32, tag="zm")
            nc.vector.tensor_add(zm[:], m1e[:], m2e[:])
            idxf = tl(rt, [P, TN], F32, tag="idxf")
            nc.vector.tensor_add(idxf[:], ia_[:], ib_[:])
            eg_ = tl(rt, [P, TN], F32, tag="eg_")
            nc.vector.tensor_scalar_add(eg_[:], egcap_t[:], -ZROW)
            nc.vector.tensor_mul(eg_[:], eg_[:], zm[:])
            nc.vector.tensor_add(idxf[:], idxf[:], eg_[:])
            nc.vector.tensor_scalar_add(idxf[:], idxf[:], ZROW)
            for e in range(NE):
                for tt in range(NTOKT):
                    nc.vector.tensor_copy(idxsel[(e, tt)][:],
                                          r3(idxf[:])[:, tt, e:e + 1])

            # slot -> source-token matrix, batched over tiles
            oh1 = tl(rt, [P, NTOKT * CAP], F32, tag="oh1")
            nc.vector.tensor_tensor(
                out=oh1[:].rearrange("p (t c) -> p t c", c=CAP),
                in0=p1[:].unsqueeze(2).to_broadcast([P, NTOKT, CAP]),
                in1=iotac_t[:].rearrange("p (t c) -> p t c", c=CAP),
                op=OP.is_equal)
            oh2 = tl(rt, [P, NTOKT * CAP], F32, tag="oh2")
            nc.vector.tensor_tensor(
                out=oh2[:].rearrange("p (t c) -> p t c", c=CAP),
                in0=p2[:].unsqueeze(2).to_broadcast([P, NTOKT, CAP]),
                in1=iotac_t[:].rearrange("p (t c) -> p t c", c=CAP),
                op=OP.is_equal)
            D = tl(rt, [P, NTOKT * NE * CAP], F32, tag="D")
            nc.vector.tensor_tensor(
                out=D[:].rearrange("p (t e c) -> p t e c", e=NE, c=CAP),
                in0=r3(m1[:]).unsqueeze(3).to_broadcast([P, NTOKT, NE, CAP]),
                in1=oh1[:].rearrange("p (t c) -> p t c", c=CAP)
                    .unsqueeze(2).to_broadcast([P, NTOKT, NE, CAP]),
                op=OP.mult)
            D2 = tl(rt, [P, NTOKT * NE * CAP], F32, tag="D2")
            nc.vector.tensor_tensor(
                out=D2[:].rearrange("p (t e c) -> p t e c", e=NE, c=CAP),
                in0=r3(m2[:]).unsqueeze(3).to_broadcast([P, NTOKT, NE, CAP]),
                in1=oh2[:].rearrange("p (t c) -> p t c", c=CAP)
                    .unsqueeze(2).to_broadcast([P, NTOKT, NE, CAP]),
                op=OP.mult)
            nc.vector.tensor_add(D[:], D[:], D2[:])
            nm = tl(psR, [GPT, NTOKT * NE * CAP], F32, tag="nm")
            nc.tensor.matmul(nm[:], lhsT=nsel_t[:], rhs=D[:],
                             start=True, stop=True)
            nm_sb = tl(rt, [GPT, NTOKT * NE * CAP], F32, tag="nm_sb")
            nc.vector.tensor_copy(nm_sb[:], nm[:])
            nc.sync.dma_start(
                nmat_d[:].rearrange("t g x -> g t x"),
                nm_sb[:].rearrange("g (t x) -> g t x", x=NE * CAP))

        # slot source-row indices: one strided readback covering all
        # (expert, slot-tile) columns, then a batched add + int cast
        with tc.tile_pool(name="ip", bufs=2) as ip:
            f_ = tl(ip, [spt, NE * nslt], F32, tag="f")
            for e_ in range(NE):
                for st in range(nslt):
                    eng = nc.sync if (e_ * nslt + st) % 2 == 0 else nc.scalar
                    eng.dma_start(
                        f_[:, e_ * nslt + st:e_ * nslt + st + 1],
                        nmat_d[:][st * tpst:(st + 1) * tpst, :,
                                  e_ * CAP:(e_ + 1) * CAP])
            nc.vector.tensor_add(f_[:], f_[:], gbase10_t[:])
            nc.vector.tensor_copy(islot_i[:], f_[:])

        # y A2A result -> group-major token order (issued here so the sync
        # queue is not blocked behind the big A2A during routing)
        ybuf16 = tl(dram, [TOK, E], F16, tag="ybuf16")
        nc.sync.dma_start(
            ybuf16[:].rearrange("(l i) r -> l i r", i=c["NC"]),
            recv16[:][:, 0:E].rearrange("(i l) r -> l i r", i=c["NC"]))

        # =========================================================
        # expert FFN (fused w1/w2 per expert) + streaming combine:
        # each expert's output is gathered and accumulated into acc[tt]
        # while the next expert computes, so only the last expert's
        # combine + LN2 remain after the FFN
        # =========================================================
        eobuf = tl(dram, [NE * GCAP + 1, E], F16, tag="eobuf")
        G1 = 2
        NMG = 32 // G1
        nc.gpsimd.dma_start(ln2g_sb[:], ln2g)
        nc.gpsimd.dma_start(ln2b_sb[:], ln2b)
        with tc.tile_pool(name="einp", bufs=1) as einp, \
             tc.tile_pool(name="eintp", bufs=1) as eintp, \
             tc.tile_pool(name="htp", bufs=2) as htp, \
             tc.tile_pool(name="eop", bufs=2) as eop, \
             tc.tile_pool(name="cmb", bufs=2) as cmb, \
             tc.tile_pool(name="psF", bufs=1, space="PSUM") as psF, \
             tc.tile_pool(name="psW2", bufs=1, space="PSUM") as psW2, \
             tc.tile_pool(name="psT", bufs=2, space="PSUM") as psT:
            # reserve pass: touch every tag once so no pool grows after a
            # later pool has stacked above it (late growth deadlocks)
            for e_ in range(NE):
                for st in range(nslt):
                    tl(einp, [spt, E], F16, tag=f"g{e_}_{st}")
            for e_ in range(NE):
                for k_ in range(KT):
                    tl(eintp, [P, GCAP], F16, tag=f"einT{e_}_{k_}")
            for kh_ in range(HIDT):
                tl(htp, [P, GCAP], F16, tag=f"ht{kh_}")
            tl(eop, [P, 512], F16, tag="eo16")
            tl(cmb, [1, E], F16, tag="zr")
            tl(cmb, [P, E], F16, tag="ysb")
            tl(cmb, [P, E], F16, tag="og")
            tl(cmb, [P, E], F32, tag="sg")
            # zero row / residual-init / expert gathers are all issued
            # lazily inside the expert loop so their DMA traffic never
            # collides with the expert-0 weight prefetch burst
            def ein_gather(e_):
                for st in range(nslt):
                    g_ = tl(einp, [spt, E], F16, tag=f"g{e_}_{st}")
                    nc.gpsimd.indirect_dma_start(
                        out=g_[:], out_offset=None, in_=ybuf16[:],
                        in_offset=bass.IndirectOffsetOnAxis(
                            ap=islot_i[:, e_ * nslt + st:e_ * nslt + st + 1],
                            axis=0))
                    eins[(e_, st)] = g_

            def combine_step(esrc, tt):
                og = tl(cmb, [P, E], F16, tag="og")
                nc.gpsimd.indirect_dma_start(
                    out=og[:], out_offset=None, in_=eobuf[:],
                    in_offset=bass.IndirectOffsetOnAxis(
                        ap=idxsel[(esrc, tt)][:, :1], axis=0))
                sg = tl(cmb, [P, E], F32, tag="sg")
                nc.scalar.activation(
                    sg[:], og[:], ACT.Copy,
                    scale=gsel[:, tt * NE + esrc:tt * NE + esrc + 1])
                nc.vector.tensor_add(acc[tt][:], acc[tt][:], sg[:])

            eins = {}
            ein_gather(0)
            ein_gather(1)

            # w1 weight tiles: half0 of expert 0 up front; thereafter each
            # (expert, half) trickles in two [128,2048] tiles per mg group
            # over the first four groups of the previous half
            halves = [(e, hf) for e in range(NE) for hf in range(2)]
            w1trickle = {"next": {}}
            w1cur = {}
            for k in range(KT):
                t = tl(wf, [P, 2048], F16, tag=f"w1_{k}")
                nc.sync.dma_start(t[:], w1[0][k * P:(k + 1) * P, 0:2048])
                w1cur[k] = t

            for e in range(NE):
                # einT for this expert (just-in-time on tensor queue)
                einT = {}
                for k in range(KT):
                    t_ = tl(eintp, [P, GCAP], F16, tag=f"einT{e}_{k}")
                    for st in range(nslt):
                        tp3 = tl(psT, [P, P], F16, tag="tp3")
                        nc.tensor.transpose(tp3[:, 0:spt],
                                            eins[(e, st)][:, k * P:(k + 1) * P],
                                            ident16[0:spt, 0:spt])
                        nc.vector.tensor_copy(t_[:, st * P:st * P + spt],
                                              tp3[:, 0:spt])
                    einT[k] = t_
                hts = {}
                pw = [tl(psW2, [P, 512], F32, tag=f"pw{i}")
                      for i in range(2 * nslt)]

                def w2_block(kh):
                    w2r = tl(w2p, [P, E], F16, tag="w2r")
                    wdma(w2r[:], w2[e][kh * P:(kh + 1) * P, :])
                    for sb in range(nslt):
                        for ch in range(2):
                            nc.tensor.matmul(
                                pw[sb * 2 + ch][:],
                                lhsT=hts[kh][:, sb * P:sb * P + spt],
                                rhs=w2r[:, ch * 512:(ch + 1) * 512],
                                start=(kh == 0), stop=(kh == HIDT - 1))

                for mg in range(NMG):
                    half, hmg = mg // 8, mg % 8
                    hseq = e * 2 + half
                    if hmg == 0:
                        if hseq > 0:
                            w1cur = w1trickle["next"]
                        w1trickle["next"] = {}
                    if hseq + 1 < len(halves) and hmg < 4:
                        en, hn = halves[hseq + 1]
                        for kpre in (2 * hmg, 2 * hmg + 1):
                            t = tl(wf, [P, 2048], F16, tag=f"w1_{kpre}")
                            wdma(t[:], w1[en][kpre * P:(kpre + 1) * P,
                                              hn * 2048:(hn + 1) * 2048])
                            w1trickle["next"][kpre] = t
                    pss = [tl(psF, [P, GCAP], F32, tag=f"ps{j}")
                           for j in range(G1)]
                    for k in range(KT):
                        for j in range(G1):
                            col = (hmg * G1 + j) * P
                            nc.tensor.matmul(
                                pss[j][:], lhsT=w1cur[k][:, col:col + P],
                                rhs=einT[k][:], start=(k == 0),
                                stop=(k == KT - 1))
                    for j in range(G1):
                        kh = mg * G1 + j
                        ht_ = tl(htp, [P, GCAP], F16, tag=f"ht{kh}")
                        nc.scalar.activation(ht_[:], pss[j][:], ACT.Gelu)
                        hts[kh] = ht_
                    if mg > 0:
                        for j in range(G1):
                            w2_block((mg - 1) * G1 + j)
                    # lazily issued side work, spread across the expert
                    if e == 0 and mg == 1:
                        zr = tl(cmb, [1, E], F16, tag="zr")
                        nc.vector.memset(zr[:], 0.0)
                        nc.sync.dma_start(eobuf[NE * GCAP:NE * GCAP + 1, :],
                                          zr[:])
                    if e == 0 and 2 <= mg <= 5:
                        tt = mg - 2
                        ysb = tl(cmb, [P, E], F16, tag="ysb")
                        nc.sync.dma_start(ysb[:],
                                          ybuf16[tt * P:(tt + 1) * P, :])
                        nc.scalar.copy(acc[tt][:], ysb[:])
                    if e + 2 < NE and mg == 6:
                        ein_gather(e + 2)
                    if e >= 1 and mg in (3, 5, 7, 9):
                        combine_step(e - 1, (mg - 3) // 2)
                for j in range(G1):
                    w2_block((NMG - 1) * G1 + j)
                for sb in range(nslt):
                    for ch in range(2):
                        eo16 = tl(eop, [P, 512], F16, tag="eo16")
                        nc.vector.tensor_copy(eo16[0:spt, :],
                                              pw[sb * 2 + ch][0:spt, :])
                        nc.sync.dma_start(
                            eobuf[e * GCAP + sb * P:e * GCAP + sb * P + spt,
                                  ch * 512:(ch + 1) * 512], eo16[0:spt, :])
            # tail: combine of the last expert
            for tt in range(NTOKT):
                combine_step(NE - 1, tt)

        # =========================================================
        # LN2 -> out
        # =========================================================
        with tc.tile_pool(name="cb", bufs=2) as cb:
            for tt in range(NTOKT):
                z = acc[tt]
                mu = tl(cb, [P, 1], F32, tag="mu")
                nc.vector.reduce_sum(mu[:], z[:], axis=AX.X)
                nc.vector.tensor_scalar_mul(mu[:], mu[:], 1.0 / E)
                xc = tl(cb, [P, E], F32, tag="xc")
                nc.vector.tensor_scalar(out=xc[:], in0=z[:], scalar1=mu[:],
                                        scalar2=None, op0=OP.subtract)
                scr = tl(cb, [P, E], F32, tag="scr")
                ssq = tl(cb, [P, 1], F32, tag="ssq")
                nc.scalar.activation(scr[:], xc[:], ACT.Square, accum_out=ssq[:])
                nc.vector.tensor_scalar(out=ssq[:], in0=ssq[:], scalar1=1.0 / E,
                                        scalar2=1e-5, op0=OP.mult, op1=OP.add)
                nc.scalar.sqrt(ssq[:], ssq[:])
                rstd = tl(cb, [P, 1], F32, tag="rstd")
                nc.vector.reciprocal(rstd[:], ssq[:])
                nc.vector.tensor_scalar_mul(xc[:], xc[:], rstd[:])
                yo = tl(cb, [P, E], F32, tag="yo")
                nc.vector.tensor_mul(yo[:], xc[:], ln2g_sb[:])
                nc.vector.tensor_add(yo[:], yo[:], ln2b_sb[:])
                nc.sync.dma_start(out[tt * P:(tt + 1) * P, :], yo[:])

    nc.compile()
    return nc


# =========================================================
# host side
# =========================================================
_CACHE = {}


def host_prep(cfg, inputs):
    """Full (unsharded) inputs -> list of per-core input maps."""
    E = cfg["E"]
    x = np.asarray(inputs["x"], np.float32)
    t = np.asarray(inputs["time"], np.float32)
    shared = dict(
        wqkvT=np.ascontiguousarray(
            np.asarray(inputs["w_qkv"], np.float32).T.astype(np.float16)),
        bqk=np.ascontiguousarray(
            np.asarray(inputs["b_qkv"], np.float32)[:2 * E, None]),
        bvrep=np.ascontiguousarray(
            np.tile(np.asarray(inputs["b_qkv"], np.float32)[None, 2 * E:], (P, 1))),
        woutT=np.ascontiguousarray(
            np.asarray(inputs["w_out"], np.float32).T.astype(np.float16)),
        bout=np.ascontiguousarray(np.asarray(inputs["b_out"], np.float32)[:, None]),
        ln1g=np.ascontiguousarray(np.asarray(inputs["ln1_g"], np.float32)[:, None]),
        ln1b=np.ascontiguousarray(np.asarray(inputs["ln1_b"], np.float32)[:, None]),
        ln2grep=np.ascontiguousarray(
            np.tile(np.asarray(inputs["ln2_g"], np.float32)[None, :], (P, 1))),
        ln2brep=np.ascontiguousarray(
            np.tile(np.asarray(inputs["ln2_b"], np.float32)[None, :], (P, 1))),
        gatew=np.ascontiguousarray(
            np.asarray(inputs["ln1_g"], np.float32)[:, None]
            * np.asarray(inputs["gate_w"], np.float32)),
        gatec0=np.ascontiguousarray(
            (np.asarray(inputs["ln1_b"], np.float32)
             @ np.asarray(inputs["gate_w"], np.float32))[:, None]),
        gatec1=np.ascontiguousarray(
            (np.asarray(inputs["ln1_g"], np.float32)
             @ np.asarray(inputs["gate_w"], np.float32))[:, None]),
        w1=np.ascontiguousarray(np.asarray(inputs["w1"]).astype(np.float16)),
        w2=np.ascontiguousarray(np.asarray(inputs["w2"]).astype(np.float16)),
    )
    in_maps = []
    for cid in range(cfg["NC"]):
        m = dict(shared)
        m["xT"] = np.ascontiguousarray(x[:, cid, :].T)
        m["tcol"] = np.ascontiguousarray(t[:, cid][:, None])
        m["trep"] = np.ascontiguousarray(np.tile(t[:, cid][None, :], (P, 1)))
        in_maps.append(m)
    return in_maps


def assemble(cfg, results):
    """Per-core 'out' (TOK, E) -> full (L, B, E)."""
    L, B, E, LC = cfg["L"], cfg["B"], cfg["E"], cfg["LC"]
    full = np.empty((L, B, E), np.float32)
    for cid in range(cfg["NC"]):
        o = np.asarray(results[cid]["out"]).reshape(LC, B, E)
        full[cid * LC:(cid + 1) * LC, :, :] = o
    return full


def get_built():
    if "full" not in _CACHE:
        cfg = make_cfg(FULL)
        _CACHE["full"] = (build_bass(cfg), cfg)
    return _CACHE["full"]


def kernel(**inputs):
    nc, cfg = get_built()
    in_maps = host_prep(cfg, inputs)
    res = run_bass_kernel_spmd(nc, in_maps, core_ids=list(range(cfg["NC"])))
    return assemble(cfg, res.results)



# revision 11
# speedup vs baseline: 1.1549x; 1.1549x over previous
"""Trainium2 Bass kernel: temporal-masked MHA + top2-gated MoE layer (8 NeuronCores).

Strategy (v2):
  - data-parallel attention over B (8 batches -> 8 cores), transposed layout,
    f16 matmul inputs (psum f32), block-causal skipping (time is sorted along
    L, so the temporal mask is block-causal; the diagonal blocks still use the
    real time comparison)
  - gate logits ride a separate tiny AllToAll so top-2 routing overlaps the
    main f16 y AllToAll
  - expert FFN: w1 weight-stationary -> hT, w2 activation-stationary (hts as
    lhsT) producing eo rows directly (no output transposes); big weight DMAs
    spread across engines; double-buffered psum
"""

import math
from contextlib import ExitStack

import numpy as np

import concourse.bass as bass
import concourse.bacc as bacc
import concourse.mybir as mybir
import concourse.tile as tile
from concourse.bass_utils import run_bass_kernel_spmd
from concourse.masks import make_identity

F32 = mybir.dt.float32
F32R = mybir.dt.float32r
F16 = mybir.dt.float16
F8 = mybir.dt.float8e4
U8 = mybir.dt.uint8
I32 = mybir.dt.int32
AX = mybir.AxisListType
OP = mybir.AluOpType
ACT = mybir.ActivationFunctionType
DR = mybir.MatmulPerfMode.DoubleRow
P = 128
# fp8 scaling: ein tokens x SA, w1/w2 x SW; descale via activation input scales
SA = 16.0
SW = 256.0

FULL = dict(L=512, B=8, E=1024, H=16, HID=4096, NE=5, NC=8)


def make_cfg(d):
    c = dict(d)
    c["CAP"] = max(min(c["B"], int(c["B"] * 2.0 / c["NE"])), 4)
    c["D"] = c["E"] // c["H"]
    assert c["D"] == 64, "head dim assumed 64"
    assert c["B"] == c["NC"]
    c["LC"] = c["L"] // c["NC"]          # L-groups per core
    c["TOK"] = c["LC"] * c["B"]          # MoE tokens per core
    assert c["TOK"] % P == 0
    assert c["L"] % P == 0
    assert c["NE"] <= 8
    c["GCAP"] = c["LC"] * c["CAP"]       # slots per expert per core
    return c


def tl(pool, shape, dtype=F32, *, tag, bufs=None):
    return pool.tile(list(shape), dtype, tag=tag, name=tag, bufs=bufs)


def build_bass(c):
    nc = bacc.Bacc("TRN2", target_bir_lowering=False, debug=False,
                   num_devices=c["NC"])
    L, B, E, H, HID, NE = c["L"], c["B"], c["E"], c["H"], c["HID"], c["NE"]
    CAP, LC, TOK, GCAP = c["CAP"], c["LC"], c["TOK"], c["GCAP"]
    KT = E // P                       # k-tiles over E
    MTOK = L // P                     # token tiles (attention, per batch)
    NTOKT = TOK // P                  # token tiles (MoE)
    NHT = 2 * E // P                  # qk row tiles
    HIDT = HID // P
    GPT = P // B                      # groups per 128-token tile
    spt = min(P, GCAP)                # slots per slot-tile
    nslt = (GCAP + P - 1) // P        # slot tiles per expert
    tpst = spt // (GPT * CAP)         # token-tiles per slot-tile
    sc = 1.0 / math.sqrt(64)

    # ---- I/O ----
    dt_ = nc.dram_tensor
    xT = dt_("xT", [E, L], F32, kind="ExternalInput")[:]
    tcol = dt_("tcol", [L, 1], F32, kind="ExternalInput")[:]
    trep = dt_("trep", [P, L], F32, kind="ExternalInput")[:]
    wqkvT = dt_("wqkvT", [E, 3 * E], F16, kind="ExternalInput")[:]
    bqk = dt_("bqk", [2 * E, 1], F32, kind="ExternalInput")[:]
    bvrep = dt_("bvrep", [P, E], F32, kind="ExternalInput")[:]
    woutT = dt_("woutT", [E, E], F16, kind="ExternalInput")[:]
    bout = dt_("bout", [E, 1], F32, kind="ExternalInput")[:]
    ln1g = dt_("ln1g", [E, 1], F32, kind="ExternalInput")[:]
    ln1b = dt_("ln1b", [E, 1], F32, kind="ExternalInput")[:]
    ln2g = dt_("ln2grep", [P, E], F32, kind="ExternalInput")[:]
    ln2b = dt_("ln2brep", [P, E], F32, kind="ExternalInput")[:]
    gw = dt_("gatew", [E, NE], F32, kind="ExternalInput")[:]   # pre-scaled by ln1_g
    gc0 = dt_("gatec0", [NE, 1], F32, kind="ExternalInput")[:]  # b @ gw
    gc1v = dt_("gatec1", [NE, 1], F32, kind="ExternalInput")[:]  # g @ gw
    # fp8 pair-interleaved expert weights (uint8 I/O; bitcast to f8 on use):
    # w1p[e, p, 2*dk+two, hid] = SW * w1[e, dk*256 + two*128 + p, hid]
    # w2p[e, p, 2*t + two, d]  = SW * w2[e, t*256 + two*128 + p, d]
    w1 = dt_("w1p", [NE, P, E // P, HID], U8,
             kind="ExternalInput")[:].bitcast(F8)
    w2 = dt_("w2p", [NE, P, HID // P, E], U8,
             kind="ExternalInput")[:].bitcast(F8)
    out = dt_("out", [TOK, E], F32, kind="ExternalOutput")[:]

    # ---- host-side constant tables (baked into the NEFF) ----
    tri = np.zeros((P, P), np.float32)       # strict-lower within B-groups
    ob = np.zeros((P, P), np.float32)        # all-ones within B-groups
    for i in range(P):
        for j in range(P):
            if i // B == j // B:
                ob[i, j] = 1.0
                if i < j:
                    tri[i, j] = 1.0
    nsel = np.zeros((P, GPT), np.float32)
    for i in range(P):
        nsel[i, i // B] = float(i % B)
    iotac4 = np.tile(np.arange(CAP, dtype=np.float32), (P, NTOKT))
    iotae4 = np.tile(np.arange(NE, dtype=np.float32), (P, NTOKT))
    gbase10 = np.zeros((spt, NE * nslt), np.float32)
    for e_ in range(NE):
        for st in range(nslt):
            for p in range(spt):
                gbase10[p, e_ * nslt + st] = float(B * ((st * P + p) // CAP))
    gb2c = np.zeros((P, NTOKT), np.float32)
    for t in range(NTOKT):
        for p in range(P):
            gb2c[p, t] = float(CAP * ((t * P + p) // B))
    egcap = np.tile(np.repeat(np.arange(NE, dtype=np.float32) * GCAP, 1),
                    (P, NTOKT))  # [P, NTOKT*NE]: e*GCAP per (t,e) column
    # one-hot head-selector columns for the batched softmax denominator
    ehall = np.zeros((P, H * H), np.float16)
    for h in range(H):
        ehall[:, h * H + h] = 1.0
    # row-h selector/replicator: rep_h = sel[h].T @ recD  (row h -> 64 rows)
    selall = np.zeros((H, H * 64), np.float16)
    for h in range(H):
        selall[h, h * 64:(h + 1) * 64] = 1.0

    # round-robin engines for weight-stream DMA triggering (vector cannot
    # trigger DMAs; gpsimd's collectives/gathers all precede these in
    # program order so its queue has slack during the FFN)
    dma_engines = [nc.sync, nc.scalar, nc.gpsimd]
    _ecnt = [0]

    def wdma(dst, src):
        e = dma_engines[_ecnt[0] % len(dma_engines)]
        _ecnt[0] += 1
        e.dma_start(dst, src)

    with tile.TileContext(nc) as tc, ExitStack() as ctx:
        cst = ctx.enter_context(tc.tile_pool(name="cst", bufs=1))
        dram = ctx.enter_context(tc.tile_pool(name="dram", bufs=1, space="DRAM"))
        pB = ctx.enter_context(tc.tile_pool(name="pB", bufs=1))
        # pB is the bottom of the SBUF stack and must not grow after later
        # pools stack above it -- allocate every persistent tile up front
        ln2g_sb = tl(pB, [P, E], F32, tag="ln2g")
        ln2b_sb = tl(pB, [P, E], F32, tag="ln2b")
        acc = [tl(pB, [P, E], F32, tag=f"acc{tt}") for tt in range(NTOKT)]
        gsel = tl(pB, [P, NTOKT * NE], F32, tag="gsel")
        gca = tl(pB, [P, NTOKT], F32, tag="gca")
        gcb = tl(pB, [P, NTOKT], F32, tag="gcb")
        idxsel = {}
        for e_ in range(NE):
            for tt in range(NTOKT):
                idxsel[(e_, tt)] = tl(pB, [P, 1], I32, tag=f"ix{e_}_{tt}")
        islot_i = tl(pB, [spt, NE * nslt], I32, tag="islot_i")

        def const_tile(arr, tag):
            ap = nc.inline_tensor(np.ascontiguousarray(arr), name=tag)[:]
            t = tl(cst, list(arr.shape), F32, tag=tag)
            nc.gpsimd.dma_start(t[:], ap)
            return t

        ident = tl(cst, [P, P], F32, tag="ident")
        make_identity(nc, ident[:])
        ident16 = tl(cst, [P, P], F16, tag="ident16")
        make_identity(nc, ident16[:])
        ones_t = tl(cst, [P, 1], F32, tag="ones")
        nc.vector.memset(ones_t[:], 1.0)
        onesr_t = tl(cst, [1, P], F32, tag="onesr")
        nc.vector.memset(onesr_t[:], 1.0)
        onesr16 = tl(cst, [1, P], F16, tag="onesr16")
        nc.vector.memset(onesr16[:], 1.0)
        ones16 = tl(cst, [P, 1], F16, tag="ones16")
        nc.vector.memset(ones16[:], 1.0)
        tri_t = const_tile(tri, "tri")
        ob_t = const_tile(ob, "ob")
        nsel_t = const_tile(nsel, "nsel")
        iotac_t = const_tile(iotac4, "iotac4")
        iotae_t = const_tile(iotae4, "iotae4")
        gb2_t = const_tile(gb2c, "gb2c")
        egcap_t = const_tile(egcap, "egcap")
        gbase10_t = const_tile(gbase10, "gbase10")
        ehbig = nc.inline_tensor(ehall, name="ehall")[:]
        ehb_t = tl(cst, [P, H * H], F16, tag="ehbig")
        nc.gpsimd.dma_start(ehb_t[:], ehbig)
        eh_ts = [ehb_t[:, h * H:(h + 1) * H] for h in range(H)]
        selbig = nc.inline_tensor(selall, name="selall")[:]
        selb_t = tl(cst, [H, H * 64], F16, tag="selbig")
        nc.gpsimd.dma_start(selb_t[:], selbig)
        sel_ts = [selb_t[:, h * 64:(h + 1) * 64] for h in range(H)]
        gc0_t = tl(cst, [NE, 1], F32, tag="gc0")
        nc.gpsimd.dma_start(gc0_t[:], gc0)
        gc1_t = tl(cst, [NE, 1], F32, tag="gc1")
        nc.gpsimd.dma_start(gc1_t[:], gc1v)

        ROWW = E + 16   # y (f16) + gate logits packed as f16 hi/lo pairs
        send16 = tl(dram, [L, ROWW], F16, tag="send16")

        # =========================================================
        # PHASE A: attention for this core's batch (transposed layout, f16)
        # =========================================================
        y16 = []
        with tc.tile_pool(name="pA", bufs=1) as pA:
            # x loads lead the sync queue; small operands ride scalar so
            # the const stream on gpsimd never gates the critical path
            xt = []
            xt16 = []
            for k in range(KT):
                t = tl(pA, [P, L], F32, tag=f"xt{k}")
                nc.sync.dma_start(t[:], xT[k * P:(k + 1) * P, :])
                xt.append(t)
                t16 = tl(pA, [P, L], F16, tag=f"xt16_{k}")
                nc.vector.tensor_copy(t16[:], t[:])
                xt16.append(t16)
            bqk_t = tl(pA, [P, NHT], F32, tag="bqk")
            nc.scalar.dma_start(bqk_t[:], bqk.rearrange("(m p) o -> p (m o)", p=P))
            bv_t = []
            for nn in range(2):
                t = tl(pA, [P, 512], F32, tag=f"bv{nn}")
                nc.scalar.dma_start(t[:], bvrep[:, nn * 512:(nn + 1) * 512])
                bv_t.append(t)
            tcol_t = tl(pA, [P, MTOK], F32, tag="tcol")
            nc.scalar.dma_start(tcol_t[:], tcol.rearrange("(m p) o -> p (m o)", p=P))
            trep_t = tl(pA, [P, L], F32, tag="trep")
            nc.scalar.dma_start(trep_t[:], trep)
            gw_t = tl(pA, [P, KT * NE], F32, tag="gw")
            nc.scalar.dma_start(gw_t[:].rearrange("p (k e) -> p k e", e=NE),
                                gw.rearrange("(k p) e -> p k e", p=P))
            bout_t = tl(pA, [P, KT], F32, tag="bout")
            nc.scalar.dma_start(bout_t[:], bout.rearrange("(m p) o -> p (m o)", p=P))
            ln1g_t = tl(pA, [P, KT], F32, tag="ln1g")
            nc.scalar.dma_start(ln1g_t[:], ln1g.rearrange("(m p) o -> p (m o)", p=P))
            ln1b_t = tl(pA, [P, KT], F32, tag="ln1b")
            nc.scalar.dma_start(ln1b_t[:], ln1b.rearrange("(m p) o -> p (m o)", p=P))

            # ---- qkT = wqkv[:2E] @ x^T (f16); V token-major w/ ones col
            qk = [tl(pA, [P, L], F16, tag=f"qk{m}") for m in range(NHT)]
            vt = [tl(pA, [P, E], F16, tag=f"vt{m}") for m in range(MTOK)]
            with tc.tile_pool(name="wp", bufs=2) as wp, \
                 tc.tile_pool(name="psQ", bufs=2, space="PSUM") as psQ:
                for mg in range(NHT // 4):
                    wq = []
                    for k in range(KT):
                        t = tl(wp, [P, 512], F16, tag=f"wq{k}")
                        nc.sync.dma_start(
                            t[:], wqkvT[k * P:(k + 1) * P,
                                        mg * 512:(mg + 1) * 512])
                        wq.append(t)
                    for j in range(4):
                        m = mg * 4 + j
                        ps = tl(psQ, [P, L], F32, tag="ps")
                        for k in range(KT):
                            nc.tensor.matmul(
                                ps[:], lhsT=wq[k][:, j * P:(j + 1) * P],
                                rhs=xt16[k][:], start=(k == 0),
                                stop=(k == KT - 1))
                        nc.vector.tensor_scalar_add(qk[m][:], ps[:],
                                                    bqk_t[:, m:m + 1])
                # V: x-stationary, weight cols streamed
                wv = {}
                for nn in range(2):
                    for k in range(KT):
                        t = tl(wp, [P, 512], F16, tag=f"wv{nn}_{k}", bufs=1)
                        nc.sync.dma_start(
                            t[:], wqkvT[k * P:(k + 1) * P,
                                        2 * E + nn * 512:2 * E + (nn + 1) * 512])
                        wv[(nn, k)] = t
                for mt in range(MTOK):
                    for nn in range(2):
                        ps = tl(psQ, [P, 512], F32, tag="ps")
                        for k in range(KT):
                            nc.tensor.matmul(
                                ps[:], lhsT=xt16[k][:, mt * P:(mt + 1) * P],
                                rhs=wv[(nn, k)][:], start=(k == 0),
                                stop=(k == KT - 1))
                        nc.vector.tensor_add(
                            vt[mt][:, nn * 512:(nn + 1) * 512], ps[:],
                            bv_t[nn][:])

            # ---- diagonal temporal masks (0 / -8e9; exp scale 1/8 -> -1e9)
            maskd = [tl(pA, [P, P], F32, tag=f"md{kt}") for kt in range(MTOK)]
            for kt in range(MTOK):
                nc.vector.tensor_tensor(
                    out=maskd[kt][:],
                    in0=tcol_t[:, kt:kt + 1].to_broadcast([P, P]),
                    in1=trep_t[:, kt * P:(kt + 1) * P], op=OP.is_gt)
                nc.vector.tensor_scalar_mul(maskd[kt][:], maskd[kt][:],
                                            -1e9 / sc)

            # ---- heads (block-causal: key tile kt only attends q >= kt*P)
            # denominators for ALL heads accumulate into one [H, L] psum via
            # one-hot lhsT columns; a single batched reciprocal replaces 16
            # serial [1,L] reciprocals (3.3us each)
            attnT = [tl(pA, [P, L], F16, tag=f"at{k}") for k in range(KT)]
            avS = [tl(pA, [64, L], F16, tag=f"avS{h}") for h in range(H)]
            with tc.tile_pool(name="pp", bufs=2) as pp, \
                 tc.tile_pool(name="smp", bufs=3) as smp, \
                 tc.tile_pool(name="psS", bufs=1, space="PSUM") as psS, \
                 tc.tile_pool(name="psD", bufs=1, space="PSUM") as psDp, \
                 tc.tile_pool(name="psAV", bufs=2, space="PSUM") as psAV:
                psD = tl(psDp, [H, L], F32, tag="psD")
                for h in range(H):
                    mq, rq = (h * 64) // P, (h * 64) % P
                    mk, rk = (E + h * 64) // P, (E + h * 64) % P
                    pts = []
                    for kt in range(MTOK):
                        N = L - kt * P
                        sps = tl(psS, [P, N], F32, tag=f"sps{kt}")
                        nc.tensor.matmul(
                            sps[:],
                            lhsT=qk[mk][rk:rk + 64, kt * P:(kt + 1) * P],
                            rhs=qk[mq][rq:rq + 64, kt * P:L],
                            start=True, stop=True)
                        nc.vector.tensor_add(sps[:, 0:P], sps[:, 0:P],
                                             maskd[kt][:])
                        pt_ = tl(pp, [P, N], F16, tag=f"pt{kt}")
                        nc.scalar.activation(pt_[:], sps[:], ACT.Exp, scale=sc)
                        pts.append(pt_)
                        nc.tensor.matmul(
                            psD[:, kt * P:L], lhsT=eh_ts[h], rhs=pt_[:],
                            start=(h == 0 and kt == 0),
                            stop=(h == H - 1 and kt == MTOK - 1),
                            skip_group_check=True)
                    av = tl(psAV, [64, L], F32, tag="av")
                    for qt in range(MTOK):
                        for kt in range(qt + 1):
                            nc.tensor.matmul(
                                av[:, qt * P:(qt + 1) * P],
                                lhsT=vt[kt][:, h * 64:h * 64 + 64],
                                rhs=pts[kt][:, (qt - kt) * P:(qt - kt + 1) * P],
                                start=(kt == 0), stop=(kt == qt))
                    nc.vector.tensor_copy(avS[h][:], av[:])
                # batched normalization
                recD = tl(smp, [H, L], F32, tag="recD")
                nc.vector.reciprocal(recD[:], psD[:])
                recD16 = tl(smp, [H, L], F16, tag="recD16")
                nc.vector.tensor_copy(recD16[:], recD[:])
                for h in range(H):
                    mq, rq = (h * 64) // P, (h * 64) % P
                    rep_ps = tl(psAV, [64, L], F32, tag="repps", bufs=1)
                    nc.tensor.matmul(rep_ps[:], lhsT=sel_ts[h],
                                     rhs=recD16[:], start=True,
                                     stop=True)
                    rep = tl(smp, [64, L], F16, tag="rep")
                    nc.vector.tensor_copy(rep[:], rep_ps[:])
                    nc.vector.tensor_mul(attnT[mq][rq:rq + 64, :], avS[h][:],
                                         rep[:])

            # ---- out-proj + residual (into xt -> zT)
            with tc.tile_pool(name="wp2", bufs=2) as wp2, \
                 tc.tile_pool(name="psO", bufs=2, space="PSUM") as psO:
                for mg in range(KT // 4):
                    wo = []
                    for k in range(KT):
                        t = tl(wp2, [P, 512], F16, tag=f"wo{k}")
                        nc.sync.dma_start(
                            t[:], woutT[k * P:(k + 1) * P,
                                        mg * 512:(mg + 1) * 512])
                        wo.append(t)
                    for j in range(4):
                        m = mg * 4 + j
                        ps = tl(psO, [P, L], F32, tag="ps")
                        for k in range(KT):
                            nc.tensor.matmul(ps[:],
                                             lhsT=wo[k][:, j * P:(j + 1) * P],
                                             rhs=attnT[k][:],
                                             start=(k == 0), stop=(k == KT - 1))
                        nc.vector.tensor_scalar_add(ps[:], ps[:],
                                                    bout_t[:, m:m + 1])
                        nc.vector.tensor_add(xt[m][:], ps[:], xt[m][:])  # zT

            # ---- LN1 stats + pre-LN gate logits
            # logits = rstd*(z @ (g.*gw)) - (rstd*mu)*(g@gw) + b@gw, so the
            # gate matmul runs on z directly and the logits A2A departs
            # before the y normalization loop
            for k in range(KT):
                y16.append(tl(pA, [P, L], F16, tag=f"y16_{k}"))
            with tc.tile_pool(name="lnp", bufs=3) as lnp, \
                 tc.tile_pool(name="gp", bufs=2) as gp, \
                 tc.tile_pool(name="psG", bufs=2, space="PSUM") as psG, \
                 tc.tile_pool(name="psL", bufs=1, space="PSUM") as psL:
                mu_ps = tl(psL, [1, L], F32, tag="mu")
                sq_ps = tl(psL, [1, L], F32, tag="sq")
                for k in range(KT):
                    z16 = tl(lnp, [P, L], F16, tag="z16")
                    nc.vector.tensor_copy(z16[:], xt[k][:])
                    nc.tensor.matmul(mu_ps[:], lhsT=ones16[:], rhs=z16[:],
                                     start=(k == 0), stop=(k == KT - 1))
                    sqt = tl(lnp, [P, L], F16, tag="sqt")
                    nc.scalar.activation(sqt[:], xt[k][:], ACT.Square)
                    nc.tensor.matmul(sq_ps[:], lhsT=ones16[:], rhs=sqt[:],
                                     start=(k == 0), stop=(k == KT - 1))
                gps = tl(psG, [NE, L], F32, tag="gps", bufs=1)
                for k in range(KT):
                    nc.tensor.matmul(
                        gps[:], lhsT=gw_t[:, k * NE:(k + 1) * NE],
                        rhs=xt[k][:], start=(k == 0), stop=(k == KT - 1))
                mu_r = tl(lnp, [1, L], F32, tag="mu_r")
                nc.vector.tensor_scalar_mul(mu_r[:], mu_ps[:], 1.0 / E)
                var_r = tl(lnp, [1, L], F32, tag="var_r")
                nc.vector.tensor_scalar_mul(var_r[:], sq_ps[:], 1.0 / E)
                mu2 = tl(lnp, [1, L], F32, tag="mu2")
                nc.vector.tensor_mul(mu2[:], mu_r[:], mu_r[:])
                nc.vector.tensor_sub(var_r[:], var_r[:], mu2[:])
                nc.vector.tensor_scalar_add(var_r[:], var_r[:], 1e-5)
                nc.scalar.sqrt(var_r[:], var_r[:])
                rstd_r = tl(lnp, [1, L], F32, tag="rstd_r")
                nc.vector.reciprocal(rstd_r[:], var_r[:])
                murst = tl(lnp, [1, L], F32, tag="murst")
                nc.vector.tensor_mul(murst[:], mu_r[:], rstd_r[:])
                mur16 = tl(lnp, [1, L], F16, tag="mur16")
                nc.vector.tensor_copy(mur16[:], mu_r[:])
                rstdr16 = tl(lnp, [1, L], F16, tag="rstdr16")
                nc.vector.tensor_copy(rstdr16[:], rstd_r[:])
                murst16 = tl(lnp, [1, L], F16, tag="murst16")
                nc.vector.tensor_copy(murst16[:], murst[:])
                # gate affine correction on [NE, L] (one psum bank, reused)
                rst5_ps = tl(psG, [NE, L], F32, tag="b5", bufs=1)
                nc.tensor.matmul(rst5_ps[:], lhsT=onesr16[:, 0:NE],
                                 rhs=rstdr16[:], start=True, stop=True)
                rst5 = tl(gp, [NE, L], F32, tag="rst5sb")
                nc.vector.tensor_copy(rst5[:], rst5_ps[:])
                mrst5_ps = tl(psG, [NE, L], F32, tag="b5", bufs=1)
                nc.tensor.matmul(mrst5_ps[:], lhsT=onesr16[:, 0:NE],
                                 rhs=murst16[:], start=True, stop=True)
                mrst5 = tl(gp, [NE, L], F32, tag="mrst5sb")
                nc.vector.tensor_scalar_mul(mrst5[:], mrst5_ps[:], gc1_t[:])
                lg_sb = tl(gp, [NE, L], F32, tag="lg_sb")
                nc.vector.tensor_tensor(out=lg_sb[:], in0=gps[:], in1=rst5[:],
                                        op=OP.mult)
                nc.vector.tensor_sub(lg_sb[:], lg_sb[:], mrst5[:])
                nc.vector.tensor_scalar_add(lg_sb[:], lg_sb[:], gc0_t[:])
                # logits -> f16 hi + residual lo (reconstructs to ~1e-7)
                lghi = tl(gp, [NE, L], F16, tag="lghi")
                nc.vector.tensor_copy(lghi[:], lg_sb[:])
                lghi32 = tl(gp, [NE, L], F32, tag="lghi32")
                nc.vector.tensor_copy(lghi32[:], lghi[:])
                lglo = tl(gp, [NE, L], F16, tag="lglo")
                nc.vector.tensor_sub(lglo[:], lg_sb[:], lghi32[:])

                # ---- y16 = LN1(z) (f16 direct; one psum bank reused)
                murep_ps = tl(psL, [P, L], F32, tag="brep", bufs=1)
                nc.tensor.matmul(murep_ps[:], lhsT=onesr16[:],
                                 rhs=mur16[:], start=True, stop=True)
                mu_rep = tl(lnp, [P, L], F32, tag="mu_rep")
                nc.vector.tensor_copy(mu_rep[:], murep_ps[:])
                rsrep_ps = tl(psL, [P, L], F32, tag="brep", bufs=1)
                nc.tensor.matmul(rsrep_ps[:], lhsT=onesr16[:],
                                 rhs=rstdr16[:], start=True, stop=True)
                rstd_rep = tl(lnp, [P, L], F32, tag="rstd_rep")
                nc.vector.tensor_copy(rstd_rep[:], rsrep_ps[:])
                for k in range(KT):
                    t1 = tl(lnp, [P, L], F32, tag="t1")
                    nc.vector.tensor_sub(t1[:], xt[k][:], mu_rep[:])
                    nc.vector.tensor_mul(t1[:], t1[:], rstd_rep[:])
                    nc.vector.tensor_scalar(
                        out=y16[k][:], in0=t1[:], scalar1=ln1g_t[:, k:k + 1],
                        scalar2=ln1b_t[:, k:k + 1], op0=OP.mult, op1=OP.add)

                # ---- y + packed logits -> token-major f16 send buffer
                for ct in range(MTOK):
                    yrow = tl(gp, [P, ROWW], F16, tag="yrow")
                    for k in range(KT):
                        tpY = tl(psG, [P, P], F16, tag="tpY")
                        nc.tensor.transpose(tpY[:], y16[k][:, ct * P:(ct + 1) * P],
                                            ident16[:])
                        nc.vector.tensor_copy(yrow[:, k * P:(k + 1) * P], tpY[:])
                    tp2 = tl(psG, [P, NE], F16, tag="tp2", bufs=1)
                    nc.tensor.transpose(tp2[:, 0:NE],
                                        lghi[:, ct * P:(ct + 1) * P],
                                        ident16[0:NE, 0:NE])
                    nc.vector.tensor_copy(yrow[:, E:E + NE], tp2[:, 0:NE])
                    tp2b = tl(psG, [P, NE], F16, tag="tp2", bufs=1)
                    nc.tensor.transpose(tp2b[:, 0:NE],
                                        lglo[:, ct * P:(ct + 1) * P],
                                        ident16[0:NE, 0:NE])
                    nc.vector.tensor_copy(yrow[:, E + 8:E + 8 + NE], tp2b[:, 0:NE])
                    nc.sync.dma_start(send16[ct * P:(ct + 1) * P, :], yrow[:])

        # FFN weight pools open once attention SBUF is released (the
        # expert-0 prefetch in the FFN section streams during A2A)
        wf = ctx.enter_context(tc.tile_pool(name="wf", bufs=2))
        w2p = ctx.enter_context(tc.tile_pool(name="w2p", bufs=6))
        tl(w2p, [P, 2 * E], F8, tag="w2r")  # reserve: pool must not grow later

        # =========================================================
        # AllToAll + permute to group-major token order
        # =========================================================
        recv16 = tl(dram, [L, ROWW], F16, tag="recv16")
        nc.gpsimd.collective_compute(
            "AllToAll", OP.bypass,
            replica_groups=[list(range(c["NC"]))],
            ins=[send16[:].opt()], outs=[recv16[:].opt()])
        lgbuf = tl(dram, [TOK, 16], F16, tag="lgbuf")
        nc.sync.dma_start(
            lgbuf[:].rearrange("(l i) r -> l i r", i=c["NC"]),
            recv16[:][:, E:E + 16].rearrange("(i l) r -> l i r", i=c["NC"]))

        # =========================================================
        # PHASE B: top-2 routing with capacity (overlaps the y A2A)
        # all 4 token-tiles processed as one [P, 4*NE] batch via 3D APs
        # =========================================================
        TN = NTOKT * NE
        nmat_d = tl(dram, [NTOKT, GPT, NE * CAP], F32, tag="nmat_d")

        def r3(ap):
            return ap.rearrange("p (t e) -> p t e", e=NE)

        with tc.tile_pool(name="rt", bufs=1) as rt, \
             tc.tile_pool(name="psR", bufs=1, space="PSUM") as psR:
            lg = tl(rt, [P, TN], F32, tag="lg")
            for tt in range(NTOKT):
                lgp = tl(rt, [P, 16], F16, tag="lgp", bufs=4)
                nc.scalar.dma_start(lgp[:], lgbuf[tt * P:(tt + 1) * P, :])
                nc.vector.tensor_add(lg[:, tt * NE:(tt + 1) * NE],
                                     lgp[:, 0:NE], lgp[:, 8:8 + NE])
            # softmax over NE per tile (logits are small: no max-sub needed)
            ex = tl(rt, [P, TN], F32, tag="ex")
            nc.scalar.activation(ex[:], lg[:], ACT.Exp)
            sm = tl(rt, [P, NTOKT], F32, tag="sm")
            nc.vector.reduce_sum(sm[:], r3(ex[:]), axis=AX.X)
            rcp = tl(rt, [P, NTOKT], F32, tag="rcp")
            nc.vector.reciprocal(rcp[:], sm[:])
            raw = tl(rt, [P, TN], F32, tag="raw")
            nc.vector.tensor_tensor(
                out=r3(raw[:]), in0=r3(ex[:]),
                in1=rcp[:].unsqueeze(2).to_broadcast([P, NTOKT, NE]),
                op=OP.mult)

            def top1(rawt, tag):
                g = tl(rt, [P, NTOKT], F32, tag=f"g{tag}")
                nc.vector.reduce_max(g[:], r3(rawt), axis=AX.X)
                eq = tl(rt, [P, TN], F32, tag=f"eq{tag}")
                nc.vector.tensor_tensor(
                    out=r3(eq[:]), in0=r3(rawt),
                    in1=g[:].unsqueeze(2).to_broadcast([P, NTOKT, NE]),
                    op=OP.is_ge)
                cs = tl(rt, [P, TN], F32, tag=f"cs{tag}")
                nc.vector.memset(r3(cs[:])[:, :, 0:1], 0.0)
                for j in range(1, NE):
                    nc.vector.tensor_add(r3(cs[:])[:, :, j:j + 1],
                                         r3(cs[:])[:, :, j - 1:j],
                                         r3(eq[:])[:, :, j - 1:j])
                fst = tl(rt, [P, TN], F32, tag=f"fst{tag}")
                nc.vector.tensor_scalar(out=fst[:], in0=cs[:], scalar1=0.5,
                                        scalar2=None, op0=OP.is_lt)
                m_ = tl(rt, [P, TN], F32, tag=f"m{tag}")
                nc.vector.tensor_mul(m_[:], eq[:], fst[:])
                return g, m_

            g1, m1r = top1(raw[:], "1")
            raw2 = tl(rt, [P, TN], F32, tag="raw2")
            nc.vector.tensor_mul(raw2[:], raw[:], m1r[:])
            nc.vector.tensor_sub(raw2[:], raw[:], raw2[:])
            g2, m2r = top1(raw2[:], "2")
            den = tl(rt, [P, NTOKT], F32, tag="den")
            nc.vector.tensor_add(den[:], g1[:], g2[:])
            nc.vector.tensor_scalar_add(den[:], den[:], 1e-9)
            rd = tl(rt, [P, NTOKT], F32, tag="rd")
            nc.vector.reciprocal(rd[:], den[:])
            g1n = tl(rt, [P, NTOKT], F32, tag="g1n")
            nc.vector.tensor_mul(g1n[:], g1[:], rd[:])
            g2n = tl(rt, [P, NTOKT], F32, tag="g2n")
            nc.vector.tensor_mul(g2n[:], g2[:], rd[:])

            # capacity by position within group (cumsum over tokens = tri/ob
            # matmuls; batched over all 4 tiles)
            pos1 = tl(psR, [P, TN], F32, tag="pos1")
            nc.tensor.matmul(pos1[:], lhsT=tri_t[:], rhs=m1r[:],
                             start=True, stop=True)
            keep1 = tl(rt, [P, TN], F32, tag="keep1")
            nc.vector.tensor_scalar(out=keep1[:], in0=pos1[:],
                                    scalar1=CAP - 0.5, scalar2=None,
                                    op0=OP.is_lt)
            m1 = tl(rt, [P, TN], F32, tag="m1k")
            nc.vector.tensor_mul(m1[:], m1r[:], keep1[:])
            pos2 = tl(psR, [P, TN], F32, tag="pos2")
            nc.tensor.matmul(pos2[:], lhsT=tri_t[:], rhs=m2r[:],
                             start=True, stop=False)
            nc.tensor.matmul(pos2[:], lhsT=ob_t[:], rhs=m1[:],
                             start=False, stop=True)
            keep2 = tl(rt, [P, TN], F32, tag="keep2")
            nc.vector.tensor_scalar(out=keep2[:], in0=pos2[:],
                                    scalar1=CAP - 0.5, scalar2=None,
                                    op0=OP.is_lt)
            m2 = tl(rt, [P, TN], F32, tag="m2k")
            nc.vector.tensor_mul(m2[:], m2r[:], keep2[:])

            def dotE(a_ap, b_ap, tag):
                t5 = tl(rt, [P, TN], F32, tag=f"t5{tag}")
                nc.vector.tensor_mul(t5[:], a_ap, b_ap)
                o = tl(rt, [P, NTOKT], F32, tag=f"o{tag}")
                nc.vector.reduce_sum(o[:], r3(t5[:]), axis=AX.X)
                return o

            m1f = tl(rt, [P, NTOKT], F32, tag="m1f")
            nc.vector.reduce_sum(m1f[:], r3(m1[:]), axis=AX.X)
            m2f = tl(rt, [P, NTOKT], F32, tag="m2f")
            nc.vector.reduce_sum(m2f[:], r3(m2[:]), axis=AX.X)
            nc.vector.tensor_mul(gca[:], g1n[:], m1f[:])
            nc.vector.tensor_mul(gcb[:], g2n[:], m2f[:])
            p1 = dotE(pos1[:], m1[:], "p1")
            p2 = dotE(pos2[:], m2[:], "p2")
            e1 = dotE(iotae_t[:], m1[:], "e1")
            e2 = dotE(iotae_t[:], m2[:], "e2")
            # capacity-dropped ranks alias to expert 0 (sums of zeroed masks);
            # bump them to a sentinel so they match no expert in m1e/m2e
            sent = tl(rt, [P, NTOKT], F32, tag="sent")
            nc.vector.tensor_scalar(out=sent[:], in0=m1f[:], scalar1=-64.0,
                                    scalar2=64.0, op0=OP.mult, op1=OP.add)
            nc.vector.tensor_add(e1[:], e1[:], sent[:])
            nc.vector.tensor_scalar(out=sent[:], in0=m2f[:], scalar1=-64.0,
                                    scalar2=64.0, op0=OP.mult, op1=OP.add)
            nc.vector.tensor_add(e2[:], e2[:], sent[:])

            # per-(expert, tile) gather indices + gate weights for the
            # streaming combine: idx = e*GCAP + group*CAP + pos if the token
            # routed to e (rank 1 or 2), else the shared zero row
            ZROW = float(NE * GCAP)
            lidx1 = tl(rt, [P, NTOKT], F32, tag="lidx1")
            nc.vector.tensor_add(lidx1[:], p1[:], gb2_t[:])
            lidx2 = tl(rt, [P, NTOKT], F32, tag="lidx2")
            nc.vector.tensor_add(lidx2[:], p2[:], gb2_t[:])
            m1e = tl(rt, [P, TN], F32, tag="m1e")
            nc.vector.tensor_tensor(
                out=r3(m1e[:]), in0=e1[:].unsqueeze(2).to_broadcast([P, NTOKT, NE]),
                in1=r3(iotae_t[:]), op=OP.is_equal)
            m2e = tl(rt, [P, TN], F32, tag="m2e")
            nc.vector.tensor_tensor(
                out=r3(m2e[:]), in0=e2[:].unsqueeze(2).to_broadcast([P, NTOKT, NE]),
                in1=r3(iotae_t[:]), op=OP.is_equal)
            ga_ = tl(rt, [P, TN], F32, tag="ga_")
            nc.vector.tensor_tensor(
                out=r3(ga_[:]), in0=gca[:].unsqueeze(2).to_broadcast([P, NTOKT, NE]),
                in1=r3(m1e[:]), op=OP.mult)
            gb_ = tl(rt, [P, TN], F32, tag="gb_")
            nc.vector.tensor_tensor(
                out=r3(gb_[:]), in0=gcb[:].unsqueeze(2).to_broadcast([P, NTOKT, NE]),
                in1=r3(m2e[:]), op=OP.mult)
            nc.vector.tensor_add(gsel[:], ga_[:], gb_[:])
            ia_ = tl(rt, [P, TN], F32, tag="ia_")
            nc.vector.tensor_tensor(
                out=r3(ia_[:]), in0=lidx1[:].unsqueeze(2).to_broadcast([P, NTOKT, NE]),
                in1=r3(m1e[:]), op=OP.mult)
            ib_ = tl(rt, [P, TN], F32, tag="ib_")
            nc.vector.tensor_tensor(
                out=r3(ib_[:]), in0=lidx2[:].unsqueeze(2).to_broadcast([P, NTOKT, NE]),
                in1=r3(m2e[:]), op=OP.mult)
            # idx = (lidx1+e*G)*m1e + (lidx2+e*G)*m2e + ZROW*(1-m1e-m2e)
            # built as: (lidx1*m1e + lidx2*m2e) + e*G*(m1e+m2e) + ZROW*(1-..)
            zm = tl(rt, [P, TN], F32, tag="zm")
            nc.vector.tensor_add(zm[:], m1e[:], m2e[:])
            idxf = tl(rt, [P, TN], F32, tag="idxf")
            nc.vector.tensor_add(idxf[:], ia_[:], ib_[:])
            eg_ = tl(rt, [P, TN], F32, tag="eg_")
            nc.vector.tensor_scalar_add(eg_[:], egcap_t[:], -ZROW)
            nc.vector.tensor_mul(eg_[:], eg_[:], zm[:])
            nc.vector.tensor_add(idxf[:], idxf[:], eg_[:])
            nc.vector.tensor_scalar_add(idxf[:], idxf[:], ZROW)
            for e in range(NE):
                for tt in range(NTOKT):
                    nc.vector.tensor_copy(idxsel[(e, tt)][:],
                                          r3(idxf[:])[:, tt, e:e + 1])

            # slot -> source-token matrix, batched over tiles
            oh1 = tl(rt, [P, NTOKT * CAP], F32, tag="oh1")
            nc.vector.tensor_tensor(
                out=oh1[:].rearrange("p (t c) -> p t c", c=CAP),
                in0=p1[:].unsqueeze(2).to_broadcast([P, NTOKT, CAP]),
                in1=iotac_t[:].rearrange("p (t c) -> p t c", c=CAP),
                op=OP.is_equal)
            oh2 = tl(rt, [P, NTOKT * CAP], F32, tag="oh2")
            nc.vector.tensor_tensor(
                out=oh2[:].rearrange("p (t c) -> p t c", c=CAP),
                in0=p2[:].unsqueeze(2).to_broadcast([P, NTOKT, CAP]),
                in1=iotac_t[:].rearrange("p (t c) -> p t c", c=CAP),
                op=OP.is_equal)
            D = tl(rt, [P, NTOKT * NE * CAP], F32, tag="D")
            nc.vector.tensor_tensor(
                out=D[:].rearrange("p (t e c) -> p t e c", e=NE, c=CAP),
                in0=r3(m1[:]).unsqueeze(3).to_broadcast([P, NTOKT, NE, CAP]),
                in1=oh1[:].rearrange("p (t c) -> p t c", c=CAP)
                    .unsqueeze(2).to_broadcast([P, NTOKT, NE, CAP]),
                op=OP.mult)
            D2 = tl(rt, [P, NTOKT * NE * CAP], F32, tag="D2")
            nc.vector.tensor_tensor(
                out=D2[:].rearrange("p (t e c) -> p t e c", e=NE, c=CAP),
                in0=r3(m2[:]).unsqueeze(3).to_broadcast([P, NTOKT, NE, CAP]),
                in1=oh2[:].rearrange("p (t c) -> p t c", c=CAP)
                    .unsqueeze(2).to_broadcast([P, NTOKT, NE, CAP]),
                op=OP.mult)
            nc.vector.tensor_add(D[:], D[:], D2[:])
            nm = tl(psR, [GPT, NTOKT * NE * CAP], F32, tag="nm")
            nc.tensor.matmul(nm[:], lhsT=nsel_t[:], rhs=D[:],
                             start=True, stop=True)
            nm_sb = tl(rt, [GPT, NTOKT * NE * CAP], F32, tag="nm_sb")
            nc.vector.tensor_copy(nm_sb[:], nm[:])
            nc.sync.dma_start(
                nmat_d[:].rearrange("t g x -> g t x"),
                nm_sb[:].rearrange("g (t x) -> g t x", x=NE * CAP))

        # slot source-row indices: one strided readback covering all
        # (expert, slot-tile) columns, then a batched add + int cast
        with tc.tile_pool(name="ip", bufs=2) as ip:
            f_ = tl(ip, [spt, NE * nslt], F32, tag="f")
            for e_ in range(NE):
                for st in range(nslt):
                    eng = nc.sync if (e_ * nslt + st) % 2 == 0 else nc.scalar
                    eng.dma_start(
                        f_[:, e_ * nslt + st:e_ * nslt + st + 1],
                        nmat_d[:][st * tpst:(st + 1) * tpst, :,
                                  e_ * CAP:(e_ + 1) * CAP])
            nc.vector.tensor_add(f_[:], f_[:], gbase10_t[:])
            nc.vector.tensor_copy(islot_i[:], f_[:])

        # y A2A result -> group-major token order (issued here so the sync
        # queue is not blocked behind the big A2A during routing)
        ybuf16 = tl(dram, [TOK, E], F16, tag="ybuf16")
        nc.sync.dma_start(
            ybuf16[:].rearrange("(l i) r -> l i r", i=c["NC"]),
            recv16[:][:, 0:E].rearrange("(i l) r -> l i r", i=c["NC"]))

        # =========================================================
        # expert FFN (fused w1/w2 per expert) + streaming combine:
        # each expert's output is gathered and accumulated into acc[tt]
        # while the next expert computes, so only the last expert's
        # combine + LN2 remain after the FFN
        # =========================================================
        eobuf = tl(dram, [NE * GCAP + 1, E], F16, tag="eobuf")
        NB = HIDT            # 32 hid-col blocks of 128
        NDT = HIDT // 2      # 16 double-k tiles over HID (for w2)
        KDT = KT // 2        # 4 double-k tiles over E (for w1)
        nc.gpsimd.dma_start(ln2g_sb[:], ln2g)
        nc.gpsimd.dma_start(ln2b_sb[:], ln2b)
        with tc.tile_pool(name="einp", bufs=1) as einp, \
             tc.tile_pool(name="eintp", bufs=1) as eintp, \
             tc.tile_pool(name="htp", bufs=2) as htp, \
             tc.tile_pool(name="eop", bufs=2) as eop, \
             tc.tile_pool(name="cmb", bufs=2) as cmb, \
             tc.tile_pool(name="psF", bufs=1, space="PSUM") as psF, \
             tc.tile_pool(name="psW2", bufs=1, space="PSUM") as psW2, \
             tc.tile_pool(name="psT", bufs=2, space="PSUM") as psT:
            # reserve pass: touch every tag once so no pool grows after a
            # later pool has stacked above it (late growth deadlocks)
            for e_ in range(NE):
                for st in range(nslt):
                    tl(einp, [spt, E], F16, tag=f"g{e_}_{st}")
            for e_ in range(NE):
                tl(eintp, [P, KT * GCAP], F8, tag=f"einT{e_}")
            tl(htp, [P, HIDT * GCAP], F8, tag="hts8")
            tl(eop, [P, 512], F16, tag="eo16")
            tl(cmb, [1, E], F16, tag="zr")
            tl(cmb, [P, E], F16, tag="ysb")
            tl(cmb, [P, E], F16, tag="og")
            tl(cmb, [P, E], F32, tag="sg")
            # zero row / residual-init / expert gathers are all issued
            # lazily inside the expert loop so their DMA traffic never
            # collides with the expert-0 weight prefetch burst
            def ein_gather(e_):
                for st in range(nslt):
                    g_ = tl(einp, [spt, E], F16, tag=f"g{e_}_{st}")
                    nc.gpsimd.indirect_dma_start(
                        out=g_[:], out_offset=None, in_=ybuf16[:],
                        in_offset=bass.IndirectOffsetOnAxis(
                            ap=islot_i[:, e_ * nslt + st:e_ * nslt + st + 1],
                            axis=0))
                    eins[(e_, st)] = g_

            def combine_step(esrc, tt):
                og = tl(cmb, [P, E], F16, tag="og")
                nc.gpsimd.indirect_dma_start(
                    out=og[:], out_offset=None, in_=eobuf[:],
                    in_offset=bass.IndirectOffsetOnAxis(
                        ap=idxsel[(esrc, tt)][:, :1], axis=0))
                sg = tl(cmb, [P, E], F32, tag="sg")
                nc.scalar.activation(
                    sg[:], og[:], ACT.Copy,
                    scale=gsel[:, tt * NE + esrc:tt * NE + esrc + 1])
                nc.vector.tensor_add(acc[tt][:], acc[tt][:], sg[:])

            eins = {}
            ein_gather(0)
            ein_gather(1)

            # expert-0 fp8 w1 [P, KT, HID] + first w2 tiles stream up front
            w1cur = tl(wf, [P, KT * HID], F8, tag="w1t")
            for c in range(KT):
                nc.sync.dma_start(
                    w1cur[:, c * HID:(c + 1) * HID], w1[0][:, c, :])
            w2tiles = {}
            w2_issued = [0]

            def w2_ensure(upto):
                while w2_issued[0] < min(upto, NE * NDT):
                    gi = w2_issued[0]
                    w2r = tl(w2p, [P, 2 * E], F8, tag="w2r")
                    wdma(w2r[:].rearrange("p (a b) -> p a b", b=E),
                         w2[gi // NDT][:, 2 * (gi % NDT):2 * (gi % NDT) + 2, :])
                    w2tiles[gi] = w2r
                    w2_issued[0] += 1

            w2_ensure(3)

            for e in range(NE):
                # einT8 for this expert: f16 transpose -> fp8 cast (x SA)
                einT8 = tl(eintp, [P, KT * GCAP], F8, tag=f"einT{e}")
                e3 = einT8[:].rearrange("p (k g) -> p k g", g=GCAP)
                for k in range(KT):
                    tp3 = tl(psT, [P, GCAP], F16, tag="tp3")
                    for st in range(nslt):
                        nc.tensor.transpose(tp3[:, st * P:st * P + spt],
                                            eins[(e, st)][:, k * P:(k + 1) * P],
                                            ident16[0:spt, 0:spt])
                    nc.scalar.activation(einT8[:, k * GCAP:(k + 1) * GCAP],
                                         tp3[:], ACT.Copy, scale=SA)
                hts8 = tl(htp, [P, HIDT * GCAP], F8, tag="hts8")
                h3 = hts8[:].rearrange("p (b g) -> p b g", g=GCAP)
                w13 = w1cur[:].rearrange("p (k h) -> p k h", h=HID)
                pw = [tl(psW2, [P, 512], F32, tag=f"pw{i}")
                      for i in range(2 * nslt)]

                def mm2_t(t_):
                    w2r3 = w2tiles.pop(e * NDT + t_)[:].rearrange(
                        "p (a b) -> p a b", b=E)
                    for sb in range(nslt):
                        for ch in range(2):
                            nc.tensor.matmul(
                                pw[sb * 2 + ch][:],
                                lhsT=h3[:, 2 * t_:2 * t_ + 2,
                                        sb * P:sb * P + spt],
                                rhs=w2r3[:, :, ch * 512:(ch + 1) * 512],
                                start=(t_ == 0), stop=(t_ == NDT - 1),
                                perf_mode=DR)

                w1nxt = None
                for b in range(NB):
                    ps = tl(psF, [P, GCAP], F32, tag=f"ps{b % 2}")
                    for dk in range(KDT):
                        nc.tensor.matmul(
                            ps[:],
                            lhsT=w13[:, 2 * dk:2 * dk + 2, b * P:(b + 1) * P],
                            rhs=e3[:, 2 * dk:2 * dk + 2, :],
                            start=(dk == 0), stop=(dk == KDT - 1),
                            perf_mode=DR)
                    nc.scalar.activation(hts8[:, b * GCAP:(b + 1) * GCAP],
                                         ps[:], ACT.Gelu, scale=1.0 / (SA * SW))
                    if b >= 2 and b % 2 == 0:
                        mm2_t((b - 2) // 2)
                        w2_ensure(e * NDT + (b - 2) // 2 + 5)
                    # next expert's w1 trickles in 8 chunks
                    if e + 1 < NE and b >= 4 and (b - 4) % 3 == 0 \
                            and (b - 4) // 3 < KT:
                        c = (b - 4) // 3
                        if w1nxt is None:
                            w1nxt = tl(wf, [P, KT * HID], F8, tag="w1t")
                        wdma(w1nxt[:, c * HID:(c + 1) * HID], w1[e + 1][:, c, :])
                    # lazily issued side work, spread across the expert
                    if e == 0 and b == 2:
                        zr = tl(cmb, [1, E], F16, tag="zr")
                        nc.vector.memset(zr[:], 0.0)
                        nc.sync.dma_start(eobuf[NE * GCAP:NE * GCAP + 1, :],
                                          zr[:])
                    if e == 0 and b in (4, 6, 8, 10):
                        tt = (b - 4) // 2
                        ysb = tl(cmb, [P, E], F16, tag="ysb")
                        nc.sync.dma_start(ysb[:],
                                          ybuf16[tt * P:(tt + 1) * P, :])
                        nc.scalar.copy(acc[tt][:], ysb[:])
                    if e + 2 < NE and b == 12:
                        ein_gather(e + 2)
                    if e >= 1 and b in (6, 10, 14, 18):
                        combine_step(e - 1, (b - 6) // 4)
                mm2_t(NDT - 1)
                if e + 1 < NE:
                    w1cur = w1nxt
                for sb in range(nslt):
                    for ch in range(2):
                        eo16 = tl(eop, [P, 512], F16, tag="eo16")
                        nc.vector.tensor_scalar_mul(
                            eo16[0:spt, :], pw[sb * 2 + ch][0:spt, :], 1.0 / SW)
                        nc.sync.dma_start(
                            eobuf[e * GCAP + sb * P:e * GCAP + sb * P + spt,
                                  ch * 512:(ch + 1) * 512], eo16[0:spt, :])
            # tail: combine of the last expert
            for tt in range(NTOKT):
                combine_step(NE - 1, tt)

        # =========================================================
        # LN2 -> out
        # =========================================================
        with tc.tile_pool(name="cb", bufs=2) as cb:
            for tt in range(NTOKT):
                z = acc[tt]
                mu = tl(cb, [P, 1], F32, tag="mu")
                nc.vector.reduce_sum(mu[:], z[:], axis=AX.X)
                nc.vector.tensor_scalar_mul(mu[:], mu[:], 1.0 / E)
                xc = tl(cb, [P, E], F32, tag="xc")
                nc.vector.tensor_scalar(out=xc[:], in0=z[:], scalar1=mu[:],
                                        scalar2=None, op0=OP.subtract)
                scr = tl(cb, [P, E], F32, tag="scr")
                ssq = tl(cb, [P, 1], F32, tag="ssq")
                nc.scalar.activation(scr[:], xc[:], ACT.Square, accum_out=ssq[:])
                nc.vector.tensor_scalar(out=ssq[:], in0=ssq[:], scalar1=1.0 / E,
                                        scalar2=1e-5, op0=OP.mult, op1=OP.add)
                nc.scalar.sqrt(ssq[:], ssq[:])
                rstd = tl(cb, [P, 1], F32, tag="rstd")
                nc.vector.reciprocal(rstd[:], ssq[:])
                nc.vector.tensor_scalar_mul(xc[:], xc[:], rstd[:])
                yo = tl(cb, [P, E], F32, tag="yo")
                nc.vector.tensor_mul(yo[:], xc[:], ln2g_sb[:])
                nc.vector.tensor_add(yo[:], yo[:], ln2b_sb[:])
                nc.sync.dma_start(out[tt * P:(tt + 1) * P, :], yo[:])

    nc.compile()
    return nc


# =========================================================
# host side
# =========================================================
_CACHE = {}


def host_prep(cfg, inputs):
    """Full (unsharded) inputs -> list of per-core input maps."""
    import ml_dtypes
    E4M3 = np.dtype(ml_dtypes.float8_e4m3)
    E, HID, NE = cfg["E"], cfg["HID"], cfg["NE"]
    x = np.asarray(inputs["x"], np.float32)
    t = np.asarray(inputs["time"], np.float32)
    # fp8 pair-interleave: [rows, cols] -> [128, rows/128, cols] with row
    # r = dk*256 + two*128 + p stored at [p, 2*dk+two, :], scaled by SW
    def pack8(w):
        r, c = w.shape
        return np.ascontiguousarray(
            (w * SW).reshape(r // 128, 128, c).transpose(1, 0, 2)
            .astype(E4M3)).view(np.uint8)
    w1_8 = np.stack([pack8(np.asarray(inputs["w1"][e], np.float32))
                     for e in range(NE)])
    w2_8 = np.stack([pack8(np.asarray(inputs["w2"][e], np.float32))
                     for e in range(NE)])
    shared = dict(
        wqkvT=np.ascontiguousarray(
            np.asarray(inputs["w_qkv"], np.float32).T.astype(np.float16)),
        bqk=np.ascontiguousarray(
            np.asarray(inputs["b_qkv"], np.float32)[:2 * E, None]),
        bvrep=np.ascontiguousarray(
            np.tile(np.asarray(inputs["b_qkv"], np.float32)[None, 2 * E:], (P, 1))),
        woutT=np.ascontiguousarray(
            np.asarray(inputs["w_out"], np.float32).T.astype(np.float16)),
        bout=np.ascontiguousarray(np.asarray(inputs["b_out"], np.float32)[:, None]),
        ln1g=np.ascontiguousarray(np.asarray(inputs["ln1_g"], np.float32)[:, None]),
        ln1b=np.ascontiguousarray(np.asarray(inputs["ln1_b"], np.float32)[:, None]),
        ln2grep=np.ascontiguousarray(
            np.tile(np.asarray(inputs["ln2_g"], np.float32)[None, :], (P, 1))),
        ln2brep=np.ascontiguousarray(
            np.tile(np.asarray(inputs["ln2_b"], np.float32)[None, :], (P, 1))),
        gatew=np.ascontiguousarray(
            np.asarray(inputs["ln1_g"], np.float32)[:, None]
            * np.asarray(inputs["gate_w"], np.float32)),
        gatec0=np.ascontiguousarray(
            (np.asarray(inputs["ln1_b"], np.float32)
             @ np.asarray(inputs["gate_w"], np.float32))[:, None]),
        gatec1=np.ascontiguousarray(
            (np.asarray(inputs["ln1_g"], np.float32)
             @ np.asarray(inputs["gate_w"], np.float32))[:, None]),
        w1p=w1_8,
        w2p=w2_8,
    )
    in_maps = []
    for cid in range(cfg["NC"]):
        m = dict(shared)
        m["xT"] = np.ascontiguousarray(x[:, cid, :].T)
        m["tcol"] = np.ascontiguousarray(t[:, cid][:, None])
        m["trep"] = np.ascontiguousarray(np.tile(t[:, cid][None, :], (P, 1)))
        in_maps.append(m)
    return in_maps


def assemble(cfg, results):
    """Per-core 'out' (TOK, E) -> full (L, B, E)."""
    L, B, E, LC = cfg["L"], cfg["B"], cfg["E"], cfg["LC"]
    full = np.empty((L, B, E), np.float32)
    for cid in range(cfg["NC"]):
        o = np.asarray(results[cid]["out"]).reshape(LC, B, E)
        full[cid * LC:(cid + 1) * LC, :, :] = o
    return full


def get_built():
    if "full" not in _CACHE:
        cfg = make_cfg(FULL)
        _CACHE["full"] = (build_bass(cfg), cfg)
    return _CACHE["full"]


def kernel(**inputs):
    nc, cfg = get_built()
    in_maps = host_prep(cfg, inputs)
    res = run_bass_kernel_spmd(nc, in_maps, core_ids=list(range(cfg["NC"])))
    return assemble(cfg, res.results)



# revision 22
# speedup vs baseline: 1.4442x; 1.2505x over previous
"""Trainium2 Bass kernel: temporal-masked MHA + top2-gated MoE layer (8 NeuronCores).

Strategy (v2):
  - data-parallel attention over B (8 batches -> 8 cores), transposed layout,
    f16 matmul inputs (psum f32), block-causal skipping (time is sorted along
    L, so the temporal mask is block-causal; the diagonal blocks still use the
    real time comparison)
  - gate logits ride a separate tiny AllToAll so top-2 routing overlaps the
    main f16 y AllToAll
  - expert FFN: w1 weight-stationary -> hT, w2 activation-stationary (hts as
    lhsT) producing eo rows directly (no output transposes); big weight DMAs
    spread across engines; double-buffered psum
"""

import math
from contextlib import ExitStack

import numpy as np

import concourse.bass as bass
import concourse.bacc as bacc
import concourse.mybir as mybir
import concourse.tile as tile
from concourse.bass_utils import run_bass_kernel_spmd
from concourse.masks import make_identity

F32 = mybir.dt.float32
F32R = mybir.dt.float32r
F16 = mybir.dt.float16
F8 = mybir.dt.float8e4
U8 = mybir.dt.uint8
I32 = mybir.dt.int32
AX = mybir.AxisListType
OP = mybir.AluOpType
ACT = mybir.ActivationFunctionType
DR = mybir.MatmulPerfMode.DoubleRow
P = 128
# fp8 scaling: ein tokens x SA, w1/w2 x SW; descale via activation input scales
SA = 16.0
SW = 256.0

FULL = dict(L=512, B=8, E=1024, H=16, HID=4096, NE=5, NC=8)


def make_cfg(d):
    c = dict(d)
    c["CAP"] = max(min(c["B"], int(c["B"] * 2.0 / c["NE"])), 4)
    c["D"] = c["E"] // c["H"]
    assert c["D"] == 64, "head dim assumed 64"
    assert c["B"] == c["NC"]
    c["LC"] = c["L"] // c["NC"]          # L-groups per core
    c["TOK"] = c["LC"] * c["B"]          # MoE tokens per core
    assert c["TOK"] % P == 0
    assert c["L"] % P == 0
    assert c["NE"] <= 8
    c["GCAP"] = c["LC"] * c["CAP"]       # slots per expert per core
    return c


def tl(pool, shape, dtype=F32, *, tag, bufs=None):
    return pool.tile(list(shape), dtype, tag=tag, name=tag, bufs=bufs)


def build_bass(c):
    nc = bacc.Bacc("TRN2", target_bir_lowering=False, debug=False,
                   num_devices=c["NC"])
    L, B, E, H, HID, NE = c["L"], c["B"], c["E"], c["H"], c["HID"], c["NE"]
    CAP, LC, TOK, GCAP = c["CAP"], c["LC"], c["TOK"], c["GCAP"]
    KT = E // P                       # k-tiles over E
    MTOK = L // P                     # token tiles (attention, per batch)
    NTOKT = TOK // P                  # token tiles (MoE)
    NHT = 2 * E // P                  # qk row tiles
    HIDT = HID // P
    GPT = P // B                      # groups per 128-token tile
    spt = min(P, GCAP)                # slots per slot-tile
    nslt = (GCAP + P - 1) // P        # slot tiles per expert
    tpst = spt // (GPT * CAP)         # token-tiles per slot-tile
    sc = 1.0 / math.sqrt(64)

    # ---- I/O ----
    dt_ = nc.dram_tensor
    xT = dt_("xT", [E, L], F32, kind="ExternalInput")[:]
    tcol = dt_("tcol", [L, 1], F32, kind="ExternalInput")[:]
    trep = dt_("trep", [P, L], F32, kind="ExternalInput")[:]
    wqkvT = dt_("wqkvT", [E, 3 * E], F16, kind="ExternalInput")[:]
    bqk = dt_("bqk", [2 * E, 1], F32, kind="ExternalInput")[:]
    bvrep = dt_("bvrep", [P, E], F32, kind="ExternalInput")[:]
    woutT = dt_("woutT", [E, E], F16, kind="ExternalInput")[:]
    bout = dt_("bout", [E, 1], F32, kind="ExternalInput")[:]
    ln1g = dt_("ln1g", [E, 1], F32, kind="ExternalInput")[:]
    ln1b = dt_("ln1b", [E, 1], F32, kind="ExternalInput")[:]
    ln2g = dt_("ln2grep", [P, E], F32, kind="ExternalInput")[:]
    ln2b = dt_("ln2brep", [P, E], F32, kind="ExternalInput")[:]
    gw = dt_("gatew", [E, NE], F32, kind="ExternalInput")[:]   # pre-scaled by ln1_g
    gc0 = dt_("gatec0", [NE, 1], F32, kind="ExternalInput")[:]  # b @ gw
    gc1v = dt_("gatec1", [NE, 1], F32, kind="ExternalInput")[:]  # g @ gw
    # fp8 pair-interleaved expert weights (uint8 I/O; bitcast to f8 on use):
    # w1p[e, p, 2*dk+two, hid] = SW * w1[e, dk*256 + two*128 + p, hid]
    # w2p[e, p, 2*t + two, d]  = SW * w2[e, t*256 + two*128 + p, d]
    w1 = dt_("w1p", [NE, P, E // P, HID], U8,
             kind="ExternalInput")[:].bitcast(F8)
    w2 = dt_("w2p", [NE, P, HID // P, E], U8,
             kind="ExternalInput")[:].bitcast(F8)
    out = dt_("out", [TOK, E], F32, kind="ExternalOutput")[:]

    # ---- host-side constant tables (baked into the NEFF) ----
    tri = np.zeros((P, P), np.float32)       # strict-lower within B-groups
    ob = np.zeros((P, P), np.float32)        # all-ones within B-groups
    for i in range(P):
        for j in range(P):
            if i // B == j // B:
                ob[i, j] = 1.0
                if i < j:
                    tri[i, j] = 1.0
    nsel = np.zeros((P, GPT), np.float32)
    for i in range(P):
        nsel[i, i // B] = float(i % B)
    iotac4 = np.tile(np.arange(CAP, dtype=np.float32), (P, NTOKT))
    iotae4 = np.tile(np.arange(NE, dtype=np.float32), (P, NTOKT))
    gbase10 = np.zeros((spt, NE * nslt), np.float32)
    for e_ in range(NE):
        for st in range(nslt):
            for p in range(spt):
                gbase10[p, e_ * nslt + st] = float(B * ((st * P + p) // CAP))
    gb2c = np.zeros((P, NTOKT), np.float32)
    for t in range(NTOKT):
        for p in range(P):
            gb2c[p, t] = float(CAP * ((t * P + p) // B))
    egcap = np.tile(np.repeat(np.arange(NE, dtype=np.float32) * GCAP, 1),
                    (P, NTOKT))  # [P, NTOKT*NE]: e*GCAP per (t,e) column
    # one-hot head-selector columns for the batched softmax denominator
    ehall = np.zeros((P, H * H), np.float16)
    for h in range(H):
        ehall[:, h * H + h] = 1.0
    # row-h selector/replicator: rep_h = sel[h].T @ recD  (row h -> 64 rows)
    selall = np.zeros((H, H * 64), np.float16)
    for h in range(H):
        selall[h, h * 64:(h + 1) * 64] = 1.0

    # round-robin engines for weight-stream DMA triggering (vector cannot
    # trigger DMAs; scalar is excluded -- its FIFO must stay clear for the
    # gelu stream that paces the FFN)
    dma_engines = [nc.sync, nc.gpsimd]
    _ecnt = [0]

    def wdma(dst, src):
        e = dma_engines[_ecnt[0] % len(dma_engines)]
        _ecnt[0] += 1
        e.dma_start(dst, src)

    with tile.TileContext(nc) as tc, ExitStack() as ctx:
        cst = ctx.enter_context(tc.tile_pool(name="cst", bufs=1))
        dram = ctx.enter_context(tc.tile_pool(name="dram", bufs=1, space="DRAM"))
        pB = ctx.enter_context(tc.tile_pool(name="pB", bufs=1))
        # pB is the bottom of the SBUF stack and must not grow after later
        # pools stack above it -- allocate every persistent tile up front
        ln2g_sb = tl(pB, [P, E], F32, tag="ln2g")
        ln2b_sb = tl(pB, [P, E], F32, tag="ln2b")
        acc = [tl(pB, [P, E], F32, tag=f"acc{tt}") for tt in range(NTOKT)]
        gsel = tl(pB, [P, NTOKT * NE], F32, tag="gsel")
        gca = tl(pB, [P, NTOKT], F32, tag="gca")
        gcb = tl(pB, [P, NTOKT], F32, tag="gcb")
        idxsel = {}
        for e_ in range(NE):
            for tt in range(NTOKT):
                idxsel[(e_, tt)] = tl(pB, [P, 1], I32, tag=f"ix{e_}_{tt}")
        islot_i = tl(pB, [spt, NE * nslt], I32, tag="islot_i")

        def const_tile(arr, tag):
            ap = nc.inline_tensor(np.ascontiguousarray(arr), name=tag)[:]
            t = tl(cst, list(arr.shape), F32, tag=tag)
            nc.gpsimd.dma_start(t[:], ap)
            return t

        ident = tl(cst, [P, P], F32, tag="ident")
        make_identity(nc, ident[:])
        ident16 = tl(cst, [P, P], F16, tag="ident16")
        make_identity(nc, ident16[:])
        ones_t = tl(cst, [P, 1], F32, tag="ones")
        nc.vector.memset(ones_t[:], 1.0)
        onesr_t = tl(cst, [1, P], F32, tag="onesr")
        nc.vector.memset(onesr_t[:], 1.0)
        onesr16 = tl(cst, [1, P], F16, tag="onesr16")
        nc.vector.memset(onesr16[:], 1.0)
        ones16 = tl(cst, [P, 1], F16, tag="ones16")
        nc.vector.memset(ones16[:], 1.0)
        tri_t = const_tile(tri, "tri")
        ob_t = const_tile(ob, "ob")
        nsel_t = const_tile(nsel, "nsel")
        iotac_t = const_tile(iotac4, "iotac4")
        iotae_t = const_tile(iotae4, "iotae4")
        gb2_t = const_tile(gb2c, "gb2c")
        egcap_t = const_tile(egcap, "egcap")
        gbase10_t = const_tile(gbase10, "gbase10")
        ehbig = nc.inline_tensor(ehall, name="ehall")[:]
        ehb_t = tl(cst, [P, H * H], F16, tag="ehbig")
        nc.gpsimd.dma_start(ehb_t[:], ehbig)
        eh_ts = [ehb_t[:, h * H:(h + 1) * H] for h in range(H)]
        selbig = nc.inline_tensor(selall, name="selall")[:]
        selb_t = tl(cst, [H, H * 64], F16, tag="selbig")
        nc.gpsimd.dma_start(selb_t[:], selbig)
        sel_ts = [selb_t[:, h * 64:(h + 1) * 64] for h in range(H)]
        gc0_t = tl(cst, [NE, 1], F32, tag="gc0")
        nc.gpsimd.dma_start(gc0_t[:], gc0)
        gc1_t = tl(cst, [NE, 1], F32, tag="gc1")
        nc.gpsimd.dma_start(gc1_t[:], gc1v)

        ROWW = E
        send16 = tl(dram, [L, ROWW], F16, tag="send16")
        lgsend = tl(dram, [L, 16], F16, tag="lgsend")

        # =========================================================
        # PHASE A: attention for this core's batch (transposed layout, f16)
        # =========================================================
        y16 = []
        with tc.tile_pool(name="pA", bufs=1) as pA:
            # x loads lead the sync queue; small operands ride scalar so
            # the const stream on gpsimd never gates the critical path
            xt = []
            xt16 = []
            for k in range(KT):
                t = tl(pA, [P, L], F32, tag=f"xt{k}")
                nc.sync.dma_start(t[:], xT[k * P:(k + 1) * P, :])
                xt.append(t)
                t16 = tl(pA, [P, L], F16, tag=f"xt16_{k}")
                nc.vector.tensor_copy(t16[:], t[:])
                xt16.append(t16)
            bqk_t = tl(pA, [P, NHT], F32, tag="bqk")
            nc.scalar.dma_start(bqk_t[:], bqk.rearrange("(m p) o -> p (m o)", p=P))
            bv_t = []
            for nn in range(2):
                t = tl(pA, [P, 512], F32, tag=f"bv{nn}")
                nc.scalar.dma_start(t[:], bvrep[:, nn * 512:(nn + 1) * 512])
                bv_t.append(t)
            tcol_t = tl(pA, [P, MTOK], F32, tag="tcol")
            nc.scalar.dma_start(tcol_t[:], tcol.rearrange("(m p) o -> p (m o)", p=P))
            trep_t = tl(pA, [P, L], F32, tag="trep")
            nc.scalar.dma_start(trep_t[:], trep)
            gw_t = tl(pA, [P, KT * NE], F32, tag="gw")
            nc.scalar.dma_start(gw_t[:].rearrange("p (k e) -> p k e", e=NE),
                                gw.rearrange("(k p) e -> p k e", p=P))
            bout_t = tl(pA, [P, KT], F32, tag="bout")
            nc.scalar.dma_start(bout_t[:], bout.rearrange("(m p) o -> p (m o)", p=P))
            ln1g_t = tl(pA, [P, KT], F32, tag="ln1g")
            nc.scalar.dma_start(ln1g_t[:], ln1g.rearrange("(m p) o -> p (m o)", p=P))
            ln1b_t = tl(pA, [P, KT], F32, tag="ln1b")
            nc.scalar.dma_start(ln1b_t[:], ln1b.rearrange("(m p) o -> p (m o)", p=P))

            # ---- qkT = wqkv[:2E] @ x^T (f16); V token-major w/ ones col
            qk = [tl(pA, [P, L], F16, tag=f"qk{m}") for m in range(NHT)]
            vt = [tl(pA, [P, E], F16, tag=f"vt{m}") for m in range(MTOK)]
            with tc.tile_pool(name="wp", bufs=2) as wp, \
                 tc.tile_pool(name="psQ", bufs=2, space="PSUM") as psQ:
                for mg in range(NHT // 4):
                    wq = []
                    for k in range(KT):
                        t = tl(wp, [P, 512], F16, tag=f"wq{k}")
                        nc.sync.dma_start(
                            t[:], wqkvT[k * P:(k + 1) * P,
                                        mg * 512:(mg + 1) * 512])
                        wq.append(t)
                    for j in range(4):
                        m = mg * 4 + j
                        ps = tl(psQ, [P, L], F32, tag="ps")
                        for k in range(KT):
                            nc.tensor.matmul(
                                ps[:], lhsT=wq[k][:, j * P:(j + 1) * P],
                                rhs=xt16[k][:], start=(k == 0),
                                stop=(k == KT - 1))
                        nc.vector.tensor_scalar_add(qk[m][:], ps[:],
                                                    bqk_t[:, m:m + 1])
                # V: x-stationary, weight cols streamed
                wv = {}
                for nn in range(2):
                    for k in range(KT):
                        t = tl(wp, [P, 512], F16, tag=f"wv{nn}_{k}", bufs=1)
                        nc.sync.dma_start(
                            t[:], wqkvT[k * P:(k + 1) * P,
                                        2 * E + nn * 512:2 * E + (nn + 1) * 512])
                        wv[(nn, k)] = t
                for mt in range(MTOK):
                    for nn in range(2):
                        ps = tl(psQ, [P, 512], F32, tag="ps")
                        for k in range(KT):
                            nc.tensor.matmul(
                                ps[:], lhsT=xt16[k][:, mt * P:(mt + 1) * P],
                                rhs=wv[(nn, k)][:], start=(k == 0),
                                stop=(k == KT - 1))
                        nc.vector.tensor_add(
                            vt[mt][:, nn * 512:(nn + 1) * 512], ps[:],
                            bv_t[nn][:])

            # ---- diagonal temporal masks (0 / -8e9; exp scale 1/8 -> -1e9)
            maskd = [tl(pA, [P, P], F32, tag=f"md{kt}") for kt in range(MTOK)]
            for kt in range(MTOK):
                nc.vector.tensor_tensor(
                    out=maskd[kt][:],
                    in0=tcol_t[:, kt:kt + 1].to_broadcast([P, P]),
                    in1=trep_t[:, kt * P:(kt + 1) * P], op=OP.is_gt)
                nc.vector.tensor_scalar_mul(maskd[kt][:], maskd[kt][:],
                                            -1e9 / sc)

            # ---- heads (block-causal: key tile kt only attends q >= kt*P)
            # denominators for ALL heads accumulate into one [H, L] psum via
            # one-hot lhsT columns; a single batched reciprocal replaces 16
            # serial [1,L] reciprocals (3.3us each)
            attnT = [tl(pA, [P, L], F16, tag=f"at{k}") for k in range(KT)]
            avS = [tl(pA, [64, L], F16, tag=f"avS{h}") for h in range(H)]
            with tc.tile_pool(name="pp", bufs=2) as pp, \
                 tc.tile_pool(name="smp", bufs=3) as smp, \
                 tc.tile_pool(name="psS", bufs=1, space="PSUM") as psS, \
                 tc.tile_pool(name="psD", bufs=1, space="PSUM") as psDp, \
                 tc.tile_pool(name="psAV", bufs=2, space="PSUM") as psAV:
                psD = tl(psDp, [H, L], F32, tag="psD")
                for h in range(H):
                    mq, rq = (h * 64) // P, (h * 64) % P
                    mk, rk = (E + h * 64) // P, (E + h * 64) % P
                    pts = []
                    for kt in range(MTOK):
                        N = L - kt * P
                        sps = tl(psS, [P, N], F32, tag=f"sps{kt}")
                        nc.tensor.matmul(
                            sps[:],
                            lhsT=qk[mk][rk:rk + 64, kt * P:(kt + 1) * P],
                            rhs=qk[mq][rq:rq + 64, kt * P:L],
                            start=True, stop=True)
                        nc.vector.tensor_add(sps[:, 0:P], sps[:, 0:P],
                                             maskd[kt][:])
                        pt_ = tl(pp, [P, N], F16, tag=f"pt{kt}")
                        nc.scalar.activation(pt_[:], sps[:], ACT.Exp, scale=sc)
                        pts.append(pt_)
                        nc.tensor.matmul(
                            psD[:, kt * P:L], lhsT=eh_ts[h], rhs=pt_[:],
                            start=(h == 0 and kt == 0),
                            stop=(h == H - 1 and kt == MTOK - 1),
                            skip_group_check=True)
                    av = tl(psAV, [64, L], F32, tag="av")
                    for qt in range(MTOK):
                        for kt in range(qt + 1):
                            nc.tensor.matmul(
                                av[:, qt * P:(qt + 1) * P],
                                lhsT=vt[kt][:, h * 64:h * 64 + 64],
                                rhs=pts[kt][:, (qt - kt) * P:(qt - kt + 1) * P],
                                start=(kt == 0), stop=(kt == qt))
                    nc.vector.tensor_copy(avS[h][:], av[:])
                # batched normalization
                recD = tl(smp, [H, L], F32, tag="recD")
                nc.vector.reciprocal(recD[:], psD[:])
                recD16 = tl(smp, [H, L], F16, tag="recD16")
                nc.vector.tensor_copy(recD16[:], recD[:])
                for h in range(H):
                    mq, rq = (h * 64) // P, (h * 64) % P
                    rep_ps = tl(psAV, [64, L], F32, tag="repps", bufs=1)
                    nc.tensor.matmul(rep_ps[:], lhsT=sel_ts[h],
                                     rhs=recD16[:], start=True,
                                     stop=True)
                    rep = tl(smp, [64, L], F16, tag="rep")
                    nc.vector.tensor_copy(rep[:], rep_ps[:])
                    nc.vector.tensor_mul(attnT[mq][rq:rq + 64, :], avS[h][:],
                                         rep[:])

            # ---- out-proj + residual (into xt -> zT)
            with tc.tile_pool(name="wp2", bufs=2) as wp2, \
                 tc.tile_pool(name="psO", bufs=2, space="PSUM") as psO:
                for mg in range(KT // 4):
                    wo = []
                    for k in range(KT):
                        t = tl(wp2, [P, 512], F16, tag=f"wo{k}")
                        nc.sync.dma_start(
                            t[:], woutT[k * P:(k + 1) * P,
                                        mg * 512:(mg + 1) * 512])
                        wo.append(t)
                    for j in range(4):
                        m = mg * 4 + j
                        ps = tl(psO, [P, L], F32, tag="ps")
                        for k in range(KT):
                            nc.tensor.matmul(ps[:],
                                             lhsT=wo[k][:, j * P:(j + 1) * P],
                                             rhs=attnT[k][:],
                                             start=(k == 0), stop=(k == KT - 1))
                        nc.vector.tensor_scalar_add(ps[:], ps[:],
                                                    bout_t[:, m:m + 1])
                        nc.vector.tensor_add(xt[m][:], ps[:], xt[m][:])  # zT

            # ---- LN1 stats + pre-LN gate logits
            # logits = rstd*(z @ (g.*gw)) - (rstd*mu)*(g@gw) + b@gw, so the
            # gate matmul runs on z directly and the logits A2A departs
            # before the y normalization loop
            for k in range(KT):
                y16.append(tl(pA, [P, L], F16, tag=f"y16_{k}"))
            with tc.tile_pool(name="lnp", bufs=3) as lnp, \
                 tc.tile_pool(name="gp", bufs=2) as gp, \
                 tc.tile_pool(name="psG", bufs=2, space="PSUM") as psG, \
                 tc.tile_pool(name="psL", bufs=1, space="PSUM") as psL:
                mu_ps = tl(psL, [1, L], F32, tag="mu")
                sq_ps = tl(psL, [1, L], F32, tag="sq")
                for k in range(KT):
                    z16 = tl(lnp, [P, L], F16, tag="z16")
                    nc.vector.tensor_copy(z16[:], xt[k][:])
                    nc.tensor.matmul(mu_ps[:], lhsT=ones16[:], rhs=z16[:],
                                     start=(k == 0), stop=(k == KT - 1))
                    sqt = tl(lnp, [P, L], F16, tag="sqt")
                    nc.scalar.activation(sqt[:], xt[k][:], ACT.Square)
                    nc.tensor.matmul(sq_ps[:], lhsT=ones16[:], rhs=sqt[:],
                                     start=(k == 0), stop=(k == KT - 1))
                gps = tl(psG, [NE, L], F32, tag="gps", bufs=1)
                for k in range(KT):
                    nc.tensor.matmul(
                        gps[:], lhsT=gw_t[:, k * NE:(k + 1) * NE],
                        rhs=xt[k][:], start=(k == 0), stop=(k == KT - 1))
                mu_r = tl(lnp, [1, L], F32, tag="mu_r")
                nc.vector.tensor_scalar_mul(mu_r[:], mu_ps[:], 1.0 / E)
                var_r = tl(lnp, [1, L], F32, tag="var_r")
                nc.vector.tensor_scalar_mul(var_r[:], sq_ps[:], 1.0 / E)
                mu2 = tl(lnp, [1, L], F32, tag="mu2")
                nc.vector.tensor_mul(mu2[:], mu_r[:], mu_r[:])
                nc.vector.tensor_sub(var_r[:], var_r[:], mu2[:])
                nc.vector.tensor_scalar_add(var_r[:], var_r[:], 1e-5)
                nc.scalar.sqrt(var_r[:], var_r[:])
                rstd_r = tl(lnp, [1, L], F32, tag="rstd_r")
                nc.vector.reciprocal(rstd_r[:], var_r[:])
                murst = tl(lnp, [1, L], F32, tag="murst")
                nc.vector.tensor_mul(murst[:], mu_r[:], rstd_r[:])
                mur16 = tl(lnp, [1, L], F16, tag="mur16")
                nc.vector.tensor_copy(mur16[:], mu_r[:])
                rstdr16 = tl(lnp, [1, L], F16, tag="rstdr16")
                nc.vector.tensor_copy(rstdr16[:], rstd_r[:])
                murst16 = tl(lnp, [1, L], F16, tag="murst16")
                nc.vector.tensor_copy(murst16[:], murst[:])
                # gate affine correction on [NE, L] (one psum bank, reused)
                rst5_ps = tl(psG, [NE, L], F32, tag="b5", bufs=1)
                nc.tensor.matmul(rst5_ps[:], lhsT=onesr16[:, 0:NE],
                                 rhs=rstdr16[:], start=True, stop=True)
                rst5 = tl(gp, [NE, L], F32, tag="rst5sb")
                nc.vector.tensor_copy(rst5[:], rst5_ps[:])
                mrst5_ps = tl(psG, [NE, L], F32, tag="b5", bufs=1)
                nc.tensor.matmul(mrst5_ps[:], lhsT=onesr16[:, 0:NE],
                                 rhs=murst16[:], start=True, stop=True)
                mrst5 = tl(gp, [NE, L], F32, tag="mrst5sb")
                nc.vector.tensor_scalar_mul(mrst5[:], mrst5_ps[:], gc1_t[:])
                lg_sb = tl(gp, [NE, L], F32, tag="lg_sb")
                nc.vector.tensor_tensor(out=lg_sb[:], in0=gps[:], in1=rst5[:],
                                        op=OP.mult)
                nc.vector.tensor_sub(lg_sb[:], lg_sb[:], mrst5[:])
                nc.vector.tensor_scalar_add(lg_sb[:], lg_sb[:], gc0_t[:])
                # logits -> f16 hi + residual lo (reconstructs to ~1e-7)
                lghi = tl(gp, [NE, L], F16, tag="lghi")
                nc.vector.tensor_copy(lghi[:], lg_sb[:])
                lghi32 = tl(gp, [NE, L], F32, tag="lghi32")
                nc.vector.tensor_copy(lghi32[:], lghi[:])
                lglo = tl(gp, [NE, L], F16, tag="lglo")
                nc.vector.tensor_sub(lglo[:], lg_sb[:], lghi32[:])
                # pack logits token-major and fire the tiny logits A2A ahead
                # of the y A2A so top-2 routing overlaps the big transfer
                lgs = tl(gp, [P, MTOK * 16], F16, tag="lgs")
                for ct in range(MTOK):
                    tp2 = tl(psG, [P, NE], F16, tag="tp2", bufs=1)
                    nc.tensor.transpose(tp2[:, 0:NE],
                                        lghi[:, ct * P:(ct + 1) * P],
                                        ident16[0:NE, 0:NE])
                    nc.vector.tensor_copy(lgs[:, ct * 16:ct * 16 + NE],
                                          tp2[:, 0:NE])
                    tp2b = tl(psG, [P, NE], F16, tag="tp2", bufs=1)
                    nc.tensor.transpose(tp2b[:, 0:NE],
                                        lglo[:, ct * P:(ct + 1) * P],
                                        ident16[0:NE, 0:NE])
                    nc.vector.tensor_copy(lgs[:, ct * 16 + 8:ct * 16 + 8 + NE],
                                          tp2b[:, 0:NE])
                nc.sync.dma_start(
                    lgsend[:].rearrange("(c p) w -> p c w", p=P),
                    lgs[:].rearrange("p (c w) -> p c w", w=16))
                lgrecv = tl(dram, [L, 16], F16, tag="lgrecv")
                nc.gpsimd.collective_compute(
                    "AllToAll", OP.bypass,
                    replica_groups=[list(range(c["NC"]))],
                    ins=[lgsend[:].opt()], outs=[lgrecv[:].opt()])

                # ---- y16 = LN1(z) (f16 direct; one psum bank reused)
                murep_ps = tl(psL, [P, L], F32, tag="brep", bufs=1)
                nc.tensor.matmul(murep_ps[:], lhsT=onesr16[:],
                                 rhs=mur16[:], start=True, stop=True)
                mu_rep = tl(lnp, [P, L], F32, tag="mu_rep")
                nc.vector.tensor_copy(mu_rep[:], murep_ps[:])
                rsrep_ps = tl(psL, [P, L], F32, tag="brep", bufs=1)
                nc.tensor.matmul(rsrep_ps[:], lhsT=onesr16[:],
                                 rhs=rstdr16[:], start=True, stop=True)
                rstd_rep = tl(lnp, [P, L], F32, tag="rstd_rep")
                nc.vector.tensor_copy(rstd_rep[:], rsrep_ps[:])
                for k in range(KT):
                    t1 = tl(lnp, [P, L], F32, tag="t1")
                    nc.vector.tensor_sub(t1[:], xt[k][:], mu_rep[:])
                    nc.vector.tensor_mul(t1[:], t1[:], rstd_rep[:])
                    nc.vector.tensor_scalar(
                        out=y16[k][:], in0=t1[:], scalar1=ln1g_t[:, k:k + 1],
                        scalar2=ln1b_t[:, k:k + 1], op0=OP.mult, op1=OP.add)

                # ---- y -> token-major f16 send buffer
                for ct in range(MTOK):
                    yrow = tl(gp, [P, ROWW], F16, tag="yrow")
                    for k in range(KT):
                        tpY = tl(psG, [P, P], F16, tag="tpY")
                        nc.tensor.transpose(tpY[:], y16[k][:, ct * P:(ct + 1) * P],
                                            ident16[:])
                        nc.vector.tensor_copy(yrow[:, k * P:(k + 1) * P], tpY[:])
                    nc.sync.dma_start(send16[ct * P:(ct + 1) * P, :], yrow[:])

        # FFN weight pools open once attention SBUF is released (the
        # expert-0 prefetch in the FFN section streams during A2A)
        wf = ctx.enter_context(tc.tile_pool(name="wf", bufs=2))
        w2p = ctx.enter_context(tc.tile_pool(name="w2p", bufs=6))
        tl(w2p, [P, 2 * E], F8, tag="w2r")  # reserve: pool must not grow later

        # =========================================================
        # AllToAll + permute to group-major token order
        # =========================================================
        recv16 = tl(dram, [L, ROWW], F16, tag="recv16")
        nc.gpsimd.collective_compute(
            "AllToAll", OP.bypass,
            replica_groups=[list(range(c["NC"]))],
            ins=[send16[:].opt()], outs=[recv16[:].opt()])
        lgbuf = tl(dram, [TOK, 16], F16, tag="lgbuf")
        nc.scalar.dma_start(
            lgbuf[:].rearrange("(l i) r -> l i r", i=c["NC"]),
            lgrecv[:].rearrange("(i l) r -> l i r", i=c["NC"]))

        # =========================================================
        # PHASE B: top-2 routing with capacity (overlaps the y A2A)
        # all 4 token-tiles processed as one [P, 4*NE] batch via 3D APs
        # =========================================================
        TN = NTOKT * NE
        nmat_d = tl(dram, [NTOKT, GPT, NE * CAP], F32, tag="nmat_d")

        def r3(ap):
            return ap.rearrange("p (t e) -> p t e", e=NE)

        with tc.tile_pool(name="rt", bufs=1) as rt, \
             tc.tile_pool(name="psR", bufs=1, space="PSUM") as psR:
            lg = tl(rt, [P, TN], F32, tag="lg")
            for tt in range(NTOKT):
                lgp = tl(rt, [P, 16], F16, tag="lgp", bufs=4)
                nc.scalar.dma_start(lgp[:], lgbuf[tt * P:(tt + 1) * P, :])
                nc.vector.tensor_add(lg[:, tt * NE:(tt + 1) * NE],
                                     lgp[:, 0:NE], lgp[:, 8:8 + NE])
            # softmax over NE per tile (logits are small: no max-sub needed)
            ex = tl(rt, [P, TN], F32, tag="ex")
            nc.scalar.activation(ex[:], lg[:], ACT.Exp)
            sm = tl(rt, [P, NTOKT], F32, tag="sm")
            nc.vector.reduce_sum(sm[:], r3(ex[:]), axis=AX.X)
            rcp = tl(rt, [P, NTOKT], F32, tag="rcp")
            nc.vector.reciprocal(rcp[:], sm[:])
            raw = tl(rt, [P, TN], F32, tag="raw")
            nc.vector.tensor_tensor(
                out=r3(raw[:]), in0=r3(ex[:]),
                in1=rcp[:].unsqueeze(2).to_broadcast([P, NTOKT, NE]),
                op=OP.mult)

            def top1(rawt, tag):
                g = tl(rt, [P, NTOKT], F32, tag=f"g{tag}")
                nc.vector.reduce_max(g[:], r3(rawt), axis=AX.X)
                eq = tl(rt, [P, TN], F32, tag=f"eq{tag}")
                nc.vector.tensor_tensor(
                    out=r3(eq[:]), in0=r3(rawt),
                    in1=g[:].unsqueeze(2).to_broadcast([P, NTOKT, NE]),
                    op=OP.is_ge)
                cs = tl(rt, [P, TN], F32, tag=f"cs{tag}")
                nc.vector.memset(r3(cs[:])[:, :, 0:1], 0.0)
                for j in range(1, NE):
                    nc.vector.tensor_add(r3(cs[:])[:, :, j:j + 1],
                                         r3(cs[:])[:, :, j - 1:j],
                                         r3(eq[:])[:, :, j - 1:j])
                fst = tl(rt, [P, TN], F32, tag=f"fst{tag}")
                nc.vector.tensor_scalar(out=fst[:], in0=cs[:], scalar1=0.5,
                                        scalar2=None, op0=OP.is_lt)
                m_ = tl(rt, [P, TN], F32, tag=f"m{tag}")
                nc.vector.tensor_mul(m_[:], eq[:], fst[:])
                return g, m_

            g1, m1r = top1(raw[:], "1")
            raw2 = tl(rt, [P, TN], F32, tag="raw2")
            nc.vector.tensor_mul(raw2[:], raw[:], m1r[:])
            nc.vector.tensor_sub(raw2[:], raw[:], raw2[:])
            g2, m2r = top1(raw2[:], "2")
            den = tl(rt, [P, NTOKT], F32, tag="den")
            nc.vector.tensor_add(den[:], g1[:], g2[:])
            nc.vector.tensor_scalar_add(den[:], den[:], 1e-9)
            rd = tl(rt, [P, NTOKT], F32, tag="rd")
            nc.vector.reciprocal(rd[:], den[:])
            g1n = tl(rt, [P, NTOKT], F32, tag="g1n")
            nc.vector.tensor_mul(g1n[:], g1[:], rd[:])
            g2n = tl(rt, [P, NTOKT], F32, tag="g2n")
            nc.vector.tensor_mul(g2n[:], g2[:], rd[:])

            # capacity by position within group (cumsum over tokens = tri/ob
            # matmuls; batched over all 4 tiles)
            pos1 = tl(psR, [P, TN], F32, tag="pos1")
            nc.tensor.matmul(pos1[:], lhsT=tri_t[:], rhs=m1r[:],
                             start=True, stop=True)
            keep1 = tl(rt, [P, TN], F32, tag="keep1")
            nc.vector.tensor_scalar(out=keep1[:], in0=pos1[:],
                                    scalar1=CAP - 0.5, scalar2=None,
                                    op0=OP.is_lt)
            m1 = tl(rt, [P, TN], F32, tag="m1k")
            nc.vector.tensor_mul(m1[:], m1r[:], keep1[:])
            pos2 = tl(psR, [P, TN], F32, tag="pos2")
            nc.tensor.matmul(pos2[:], lhsT=tri_t[:], rhs=m2r[:],
                             start=True, stop=False)
            nc.tensor.matmul(pos2[:], lhsT=ob_t[:], rhs=m1[:],
                             start=False, stop=True)
            keep2 = tl(rt, [P, TN], F32, tag="keep2")
            nc.vector.tensor_scalar(out=keep2[:], in0=pos2[:],
                                    scalar1=CAP - 0.5, scalar2=None,
                                    op0=OP.is_lt)
            m2 = tl(rt, [P, TN], F32, tag="m2k")
            nc.vector.tensor_mul(m2[:], m2r[:], keep2[:])

            def dotE(a_ap, b_ap, tag):
                t5 = tl(rt, [P, TN], F32, tag=f"t5{tag}")
                nc.vector.tensor_mul(t5[:], a_ap, b_ap)
                o = tl(rt, [P, NTOKT], F32, tag=f"o{tag}")
                nc.vector.reduce_sum(o[:], r3(t5[:]), axis=AX.X)
                return o

            m1f = tl(rt, [P, NTOKT], F32, tag="m1f")
            nc.vector.reduce_sum(m1f[:], r3(m1[:]), axis=AX.X)
            m2f = tl(rt, [P, NTOKT], F32, tag="m2f")
            nc.vector.reduce_sum(m2f[:], r3(m2[:]), axis=AX.X)
            nc.vector.tensor_mul(gca[:], g1n[:], m1f[:])
            nc.vector.tensor_mul(gcb[:], g2n[:], m2f[:])
            p1 = dotE(pos1[:], m1[:], "p1")
            p2 = dotE(pos2[:], m2[:], "p2")
            e1 = dotE(iotae_t[:], m1[:], "e1")
            e2 = dotE(iotae_t[:], m2[:], "e2")
            # capacity-dropped ranks alias to expert 0 (sums of zeroed masks);
            # bump them to a sentinel so they match no expert in m1e/m2e
            sent = tl(rt, [P, NTOKT], F32, tag="sent")
            nc.vector.tensor_scalar(out=sent[:], in0=m1f[:], scalar1=-64.0,
                                    scalar2=64.0, op0=OP.mult, op1=OP.add)
            nc.vector.tensor_add(e1[:], e1[:], sent[:])
            nc.vector.tensor_scalar(out=sent[:], in0=m2f[:], scalar1=-64.0,
                                    scalar2=64.0, op0=OP.mult, op1=OP.add)
            nc.vector.tensor_add(e2[:], e2[:], sent[:])

            # per-(expert, tile) gather indices + gate weights for the
            # streaming combine: idx = e*GCAP + group*CAP + pos if the token
            # routed to e (rank 1 or 2), else the shared zero row
            ZROW = float(NE * GCAP)
            lidx1 = tl(rt, [P, NTOKT], F32, tag="lidx1")
            nc.vector.tensor_add(lidx1[:], p1[:], gb2_t[:])
            lidx2 = tl(rt, [P, NTOKT], F32, tag="lidx2")
            nc.vector.tensor_add(lidx2[:], p2[:], gb2_t[:])
            m1e = tl(rt, [P, TN], F32, tag="m1e")
            nc.vector.tensor_tensor(
                out=r3(m1e[:]), in0=e1[:].unsqueeze(2).to_broadcast([P, NTOKT, NE]),
                in1=r3(iotae_t[:]), op=OP.is_equal)
            m2e = tl(rt, [P, TN], F32, tag="m2e")
            nc.vector.tensor_tensor(
                out=r3(m2e[:]), in0=e2[:].unsqueeze(2).to_broadcast([P, NTOKT, NE]),
                in1=r3(iotae_t[:]), op=OP.is_equal)
            ga_ = tl(rt, [P, TN], F32, tag="ga_")
            nc.vector.tensor_tensor(
                out=r3(ga_[:]), in0=gca[:].unsqueeze(2).to_broadcast([P, NTOKT, NE]),
                in1=r3(m1e[:]), op=OP.mult)
            gb_ = tl(rt, [P, TN], F32, tag="gb_")
            nc.vector.tensor_tensor(
                out=r3(gb_[:]), in0=gcb[:].unsqueeze(2).to_broadcast([P, NTOKT, NE]),
                in1=r3(m2e[:]), op=OP.mult)
            nc.vector.tensor_add(gsel[:], ga_[:], gb_[:])
            ia_ = tl(rt, [P, TN], F32, tag="ia_")
            nc.vector.tensor_tensor(
                out=r3(ia_[:]), in0=lidx1[:].unsqueeze(2).to_broadcast([P, NTOKT, NE]),
                in1=r3(m1e[:]), op=OP.mult)
            ib_ = tl(rt, [P, TN], F32, tag="ib_")
            nc.vector.tensor_tensor(
                out=r3(ib_[:]), in0=lidx2[:].unsqueeze(2).to_broadcast([P, NTOKT, NE]),
                in1=r3(m2e[:]), op=OP.mult)
            # idx = (lidx1+e*G)*m1e + (lidx2+e*G)*m2e + ZROW*(1-m1e-m2e)
            # built as: (lidx1*m1e + lidx2*m2e) + e*G*(m1e+m2e) + ZROW*(1-..)
            zm = tl(rt, [P, TN], F32, tag="zm")
            nc.vector.tensor_add(zm[:], m1e[:], m2e[:])
            idxf = tl(rt, [P, TN], F32, tag="idxf")
            nc.vector.tensor_add(idxf[:], ia_[:], ib_[:])
            eg_ = tl(rt, [P, TN], F32, tag="eg_")
            nc.vector.tensor_scalar_add(eg_[:], egcap_t[:], -ZROW)
            nc.vector.tensor_mul(eg_[:], eg_[:], zm[:])
            nc.vector.tensor_add(idxf[:], idxf[:], eg_[:])
            nc.vector.tensor_scalar_add(idxf[:], idxf[:], ZROW)
            for e in range(NE):
                for tt in range(NTOKT):
                    nc.vector.tensor_copy(idxsel[(e, tt)][:],
                                          r3(idxf[:])[:, tt, e:e + 1])

            # slot -> source-token matrix, batched over tiles
            oh1 = tl(rt, [P, NTOKT * CAP], F32, tag="oh1")
            nc.vector.tensor_tensor(
                out=oh1[:].rearrange("p (t c) -> p t c", c=CAP),
                in0=p1[:].unsqueeze(2).to_broadcast([P, NTOKT, CAP]),
                in1=iotac_t[:].rearrange("p (t c) -> p t c", c=CAP),
                op=OP.is_equal)
            oh2 = tl(rt, [P, NTOKT * CAP], F32, tag="oh2")
            nc.vector.tensor_tensor(
                out=oh2[:].rearrange("p (t c) -> p t c", c=CAP),
                in0=p2[:].unsqueeze(2).to_broadcast([P, NTOKT, CAP]),
                in1=iotac_t[:].rearrange("p (t c) -> p t c", c=CAP),
                op=OP.is_equal)
            D = tl(rt, [P, NTOKT * NE * CAP], F32, tag="D")
            nc.vector.tensor_tensor(
                out=D[:].rearrange("p (t e c) -> p t e c", e=NE, c=CAP),
                in0=r3(m1[:]).unsqueeze(3).to_broadcast([P, NTOKT, NE, CAP]),
                in1=oh1[:].rearrange("p (t c) -> p t c", c=CAP)
                    .unsqueeze(2).to_broadcast([P, NTOKT, NE, CAP]),
                op=OP.mult)
            D2 = tl(rt, [P, NTOKT * NE * CAP], F32, tag="D2")
            nc.vector.tensor_tensor(
                out=D2[:].rearrange("p (t e c) -> p t e c", e=NE, c=CAP),
                in0=r3(m2[:]).unsqueeze(3).to_broadcast([P, NTOKT, NE, CAP]),
                in1=oh2[:].rearrange("p (t c) -> p t c", c=CAP)
                    .unsqueeze(2).to_broadcast([P, NTOKT, NE, CAP]),
                op=OP.mult)
            nc.vector.tensor_add(D[:], D[:], D2[:])
            nm = tl(psR, [GPT, NTOKT * NE * CAP], F32, tag="nm")
            nc.tensor.matmul(nm[:], lhsT=nsel_t[:], rhs=D[:],
                             start=True, stop=True)
            nm_sb = tl(rt, [GPT, NTOKT * NE * CAP], F32, tag="nm_sb")
            nc.vector.tensor_copy(nm_sb[:], nm[:])
            nc.sync.dma_start(
                nmat_d[:].rearrange("t g x -> g t x"),
                nm_sb[:].rearrange("g (t x) -> g t x", x=NE * CAP))

        # slot source-row indices: one strided readback covering all
        # (expert, slot-tile) columns, then a batched add + int cast
        with tc.tile_pool(name="ip", bufs=2) as ip:
            f_ = tl(ip, [spt, NE * nslt], F32, tag="f")
            for e_ in range(NE):
                for st in range(nslt):
                    eng = nc.sync if (e_ * nslt + st) % 2 == 0 else nc.scalar
                    eng.dma_start(
                        f_[:, e_ * nslt + st:e_ * nslt + st + 1],
                        nmat_d[:][st * tpst:(st + 1) * tpst, :,
                                  e_ * CAP:(e_ + 1) * CAP])
            nc.vector.tensor_add(f_[:], f_[:], gbase10_t[:])
            nc.vector.tensor_copy(islot_i[:], f_[:])

        # y A2A result -> group-major token order (issued here so the sync
        # queue is not blocked behind the big A2A during routing)
        ybuf16 = tl(dram, [TOK, E], F16, tag="ybuf16")
        nc.sync.dma_start(
            ybuf16[:].rearrange("(l i) r -> l i r", i=c["NC"]),
            recv16[:][:, 0:E].rearrange("(i l) r -> l i r", i=c["NC"]))

        # =========================================================
        # expert FFN (fused w1/w2 per expert) + streaming combine:
        # each expert's output is gathered and accumulated into acc[tt]
        # while the next expert computes, so only the last expert's
        # combine + LN2 remain after the FFN
        # =========================================================
        eobuf = tl(dram, [NE * GCAP + 1, E], F16, tag="eobuf")
        NB = HIDT            # 32 hid-col blocks of 128
        NDT = HIDT // 2      # 16 double-k tiles over HID (for w2)
        KDT = KT // 2        # 4 double-k tiles over E (for w1)
        nc.gpsimd.dma_start(ln2g_sb[:], ln2g)
        nc.gpsimd.dma_start(ln2b_sb[:], ln2b)
        with tc.tile_pool(name="einp", bufs=1) as einp, \
             tc.tile_pool(name="eintp", bufs=1) as eintp, \
             tc.tile_pool(name="htp", bufs=2) as htp, \
             tc.tile_pool(name="eop", bufs=2) as eop, \
             tc.tile_pool(name="cmb", bufs=2) as cmb, \
             tc.tile_pool(name="psF", bufs=1, space="PSUM") as psF, \
             tc.tile_pool(name="psW2", bufs=1, space="PSUM") as psW2, \
             tc.tile_pool(name="psT", bufs=2, space="PSUM") as psT:
            # reserve pass: touch every tag once so no pool grows after a
            # later pool has stacked above it (late growth deadlocks)
            for e_ in range(NE):
                for st in range(nslt):
                    tl(einp, [spt, E], F16, tag=f"g{e_}_{st}")
            for e_ in range(NE):
                tl(eintp, [P, KT * GCAP], F8, tag=f"einT{e_}")
            tl(htp, [P, HIDT * GCAP], F8, tag="hts8")
            tl(eop, [P, 512], F16, tag="eo16")
            tl(cmb, [1, E], F16, tag="zr")
            tl(cmb, [P, E], F16, tag="ysb")
            for tt_ in range(NTOKT):
                tl(cmb, [P, E], F16, tag=f"og{tt_}")
            tl(cmb, [P, E], F32, tag="sg")
            # zero row / residual-init / expert gathers are all issued
            # lazily inside the expert loop so their DMA traffic never
            # collides with the expert-0 weight prefetch burst
            def ein_gather(e_):
                for st in range(nslt):
                    g_ = tl(einp, [spt, E], F16, tag=f"g{e_}_{st}")
                    nc.gpsimd.indirect_dma_start(
                        out=g_[:], out_offset=None, in_=ybuf16[:],
                        in_offset=bass.IndirectOffsetOnAxis(
                            ap=islot_i[:, e_ * nslt + st:e_ * nslt + st + 1],
                            axis=0))
                    eins[(e_, st)] = g_

            def combine_gather(esrc, tt):
                og = tl(cmb, [P, E], F16, tag=f"og{tt}")
                nc.gpsimd.indirect_dma_start(
                    out=og[:], out_offset=None, in_=eobuf[:],
                    in_offset=bass.IndirectOffsetOnAxis(
                        ap=idxsel[(esrc, tt)][:, :1], axis=0))
                return og

            def combine_apply(og, esrc, tt):
                sg = tl(cmb, [P, E], F32, tag="sg")
                nc.vector.tensor_scalar_mul(
                    sg[:], og[:], gsel[:, tt * NE + esrc:tt * NE + esrc + 1])
                nc.vector.tensor_add(acc[tt][:], acc[tt][:], sg[:])

            def combine_step(esrc, tt):
                combine_apply(combine_gather(esrc, tt), esrc, tt)

            eins = {}
            ein_gather(0)
            ein_gather(1)

            # expert-0 fp8 w1 [P, KT, HID] + first w2 tiles stream up front
            w1cur = tl(wf, [P, KT * HID], F8, tag="w1t")
            for c in range(KT):
                nc.sync.dma_start(
                    w1cur[:, c * HID:(c + 1) * HID], w1[0][:, c, :])
            w2tiles = {}
            w2_issued = [0]

            def w2_ensure(upto):
                while w2_issued[0] < min(upto, NE * NDT):
                    gi = w2_issued[0]
                    w2r = tl(w2p, [P, 2 * E], F8, tag="w2r")
                    wdma(w2r[:].rearrange("p (a b) -> p a b", b=E),
                         w2[gi // NDT][:, 2 * (gi % NDT):2 * (gi % NDT) + 2, :])
                    w2tiles[gi] = w2r
                    w2_issued[0] += 1

            w2_ensure(3)

            for e in range(NE):
                # einT8 for this expert: f16 transpose -> fp8 cast (x SA)
                einT8 = tl(eintp, [P, KT * GCAP], F8, tag=f"einT{e}")
                e3 = einT8[:].rearrange("p (k g) -> p k g", g=GCAP)
                for k in range(KT):
                    tp3 = tl(psT, [P, GCAP], F16, tag="tp3")
                    for st in range(nslt):
                        nc.tensor.transpose(tp3[:, st * P:st * P + spt],
                                            eins[(e, st)][:, k * P:(k + 1) * P],
                                            ident16[0:spt, 0:spt])
                    nc.vector.tensor_scalar_mul(
                        einT8[:, k * GCAP:(k + 1) * GCAP], tp3[:], SA)
                hts8 = tl(htp, [P, HIDT * GCAP], F8, tag="hts8")
                h3 = hts8[:].rearrange("p (b g) -> p b g", g=GCAP)
                w13 = w1cur[:].rearrange("p (k h) -> p k h", h=HID)
                pw = [tl(psW2, [P, 512], F32, tag=f"pw{i}")
                      for i in range(2 * nslt)]

                def mm2_t(t_):
                    w2r3 = w2tiles.pop(e * NDT + t_)[:].rearrange(
                        "p (a b) -> p a b", b=E)
                    for sb in range(nslt):
                        for ch in range(2):
                            nc.tensor.matmul(
                                pw[sb * 2 + ch][:],
                                lhsT=h3[:, 2 * t_:2 * t_ + 2,
                                        sb * P:sb * P + spt],
                                rhs=w2r3[:, :, ch * 512:(ch + 1) * 512],
                                start=(t_ == 0), stop=(t_ == NDT - 1),
                                perf_mode=DR)

                w1nxt = None
                for b in range(NB):
                    ps = tl(psF, [P, GCAP], F32, tag=f"ps{b % 2}")
                    for dk in range(KDT):
                        nc.tensor.matmul(
                            ps[:],
                            lhsT=w13[:, 2 * dk:2 * dk + 2, b * P:(b + 1) * P],
                            rhs=e3[:, 2 * dk:2 * dk + 2, :],
                            start=(dk == 0), stop=(dk == KDT - 1),
                            perf_mode=DR)
                    nc.scalar.activation(hts8[:, b * GCAP:(b + 1) * GCAP],
                                         ps[:], ACT.Gelu, scale=1.0 / (SA * SW))
                    if b >= 2 and b % 2 == 0:
                        mm2_t((b - 2) // 2)
                        w2_ensure(e * NDT + (b - 2) // 2 + 5)
                    # next expert's w1 trickles in 8 chunks
                    if e + 1 < NE and b >= 4 and (b - 4) % 3 == 0 \
                            and (b - 4) // 3 < KT:
                        c = (b - 4) // 3
                        if w1nxt is None:
                            w1nxt = tl(wf, [P, KT * HID], F8, tag="w1t")
                        wdma(w1nxt[:, c * HID:(c + 1) * HID], w1[e + 1][:, c, :])
                    # lazily issued side work, spread across the expert
                    if e == 0 and b == 2:
                        zr = tl(cmb, [1, E], F16, tag="zr")
                        nc.vector.memset(zr[:], 0.0)
                        nc.sync.dma_start(eobuf[NE * GCAP:NE * GCAP + 1, :],
                                          zr[:])
                    if e == 0 and b in (4, 6, 8, 10):
                        tt = (b - 4) // 2
                        ysb = tl(cmb, [P, E], F16, tag="ysb")
                        nc.sync.dma_start(ysb[:],
                                          ybuf16[tt * P:(tt + 1) * P, :])
                        nc.vector.tensor_copy(acc[tt][:], ysb[:])
                    if e + 2 < NE and b == 12:
                        ein_gather(e + 2)
                    if e >= 1 and b in (6, 10, 14, 18):
                        combine_step(e - 1, (b - 6) // 4)
                mm2_t(NDT - 1)
                if e + 1 < NE:
                    w1cur = w1nxt
                for sb in range(nslt):
                    for ch in range(2):
                        eo16 = tl(eop, [P, 512], F16, tag="eo16")
                        nc.vector.tensor_scalar_mul(
                            eo16[0:spt, :], pw[sb * 2 + ch][0:spt, :], 1.0 / SW)
                        nc.sync.dma_start(
                            eobuf[e * GCAP + sb * P:e * GCAP + sb * P + spt,
                                  ch * 512:(ch + 1) * 512], eo16[0:spt, :])
            # tail: combine of the last expert -- all four gathers go out
            # first so the scale/add/LN2 chains pipeline behind them
            ogs = [combine_gather(NE - 1, tt) for tt in range(NTOKT)]
            for tt in range(NTOKT):
                combine_apply(ogs[tt], NE - 1, tt)

        # =========================================================
        # LN2 -> out
        # =========================================================
        with tc.tile_pool(name="cb", bufs=4) as cb:
            for tt in range(NTOKT):
                z = acc[tt]
                mu = tl(cb, [P, 1], F32, tag="mu")
                nc.vector.reduce_sum(mu[:], z[:], axis=AX.X)
                nc.vector.tensor_scalar_mul(mu[:], mu[:], 1.0 / E)
                xc = tl(cb, [P, E], F32, tag="xc")
                nc.vector.tensor_scalar(out=xc[:], in0=z[:], scalar1=mu[:],
                                        scalar2=None, op0=OP.subtract)
                scr = tl(cb, [P, E], F16, tag="scr")
                ssq = tl(cb, [P, 1], F32, tag="ssq")
                nc.scalar.activation(scr[:], xc[:], ACT.Square, accum_out=ssq[:])
                nc.vector.tensor_scalar(out=ssq[:], in0=ssq[:], scalar1=1.0 / E,
                                        scalar2=1e-5, op0=OP.mult, op1=OP.add)
                nc.scalar.sqrt(ssq[:], ssq[:])
                rstd = tl(cb, [P, 1], F32, tag="rstd")
                nc.vector.reciprocal(rstd[:], ssq[:])
                nc.vector.tensor_scalar_mul(xc[:], xc[:], rstd[:])
                yo = tl(cb, [P, E], F32, tag="yo")
                nc.vector.tensor_mul(yo[:], xc[:], ln2g_sb[:])
                nc.vector.tensor_add(yo[:], yo[:], ln2b_sb[:])
                nc.sync.dma_start(out[tt * P:(tt + 1) * P, :], yo[:])

    nc.compile()
    return nc


# =========================================================
# host side
# =========================================================
_CACHE = {}


def host_prep(cfg, inputs):
    """Full (unsharded) inputs -> list of per-core input maps."""
    import ml_dtypes
    E4M3 = np.dtype(ml_dtypes.float8_e4m3)
    E, HID, NE = cfg["E"], cfg["HID"], cfg["NE"]
    x = np.asarray(inputs["x"], np.float32)
    t = np.asarray(inputs["time"], np.float32)
    # fp8 pair-interleave: [rows, cols] -> [128, rows/128, cols] with row
    # r = dk*256 + two*128 + p stored at [p, 2*dk+two, :], scaled by SW
    def pack8(w):
        r, c = w.shape
        return np.ascontiguousarray(
            (w * SW).reshape(r // 128, 128, c).transpose(1, 0, 2)
            .astype(E4M3)).view(np.uint8)
    w1_8 = np.stack([pack8(np.asarray(inputs["w1"][e], np.float32))
                     for e in range(NE)])
    w2_8 = np.stack([pack8(np.asarray(inputs["w2"][e], np.float32))
                     for e in range(NE)])
    shared = dict(
        wqkvT=np.ascontiguousarray(
            np.asarray(inputs["w_qkv"], np.float32).T.astype(np.float16)),
        bqk=np.ascontiguousarray(
            np.asarray(inputs["b_qkv"], np.float32)[:2 * E, None]),
        bvrep=np.ascontiguousarray(
            np.tile(np.asarray(inputs["b_qkv"], np.float32)[None, 2 * E:], (P, 1))),
        woutT=np.ascontiguousarray(
            np.asarray(inputs["w_out"], np.float32).T.astype(np.float16)),
        bout=np.ascontiguousarray(np.asarray(inputs["b_out"], np.float32)[:, None]),
        ln1g=np.ascontiguousarray(np.asarray(inputs["ln1_g"], np.float32)[:, None]),
        ln1b=np.ascontiguousarray(np.asarray(inputs["ln1_b"], np.float32)[:, None]),
        ln2grep=np.ascontiguousarray(
            np.tile(np.asarray(inputs["ln2_g"], np.float32)[None, :], (P, 1))),
        ln2brep=np.ascontiguousarray(
            np.tile(np.asarray(inputs["ln2_b"], np.float32)[None, :], (P, 1))),
        gatew=np.ascontiguousarray(
            np.asarray(inputs["ln1_g"], np.float32)[:, None]
            * np.asarray(inputs["gate_w"], np.float32)),
        gatec0=np.ascontiguousarray(
            (np.asarray(inputs["ln1_b"], np.float32)
             @ np.asarray(inputs["gate_w"], np.float32))[:, None]),
        gatec1=np.ascontiguousarray(
            (np.asarray(inputs["ln1_g"], np.float32)
             @ np.asarray(inputs["gate_w"], np.float32))[:, None]),
        w1p=w1_8,
        w2p=w2_8,
    )
    in_maps = []
    for cid in range(cfg["NC"]):
        m = dict(shared)
        m["xT"] = np.ascontiguousarray(x[:, cid, :].T)
        m["tcol"] = np.ascontiguousarray(t[:, cid][:, None])
        m["trep"] = np.ascontiguousarray(np.tile(t[:, cid][None, :], (P, 1)))
        in_maps.append(m)
    return in_maps


def assemble(cfg, results):
    """Per-core 'out' (TOK, E) -> full (L, B, E)."""
    L, B, E, LC = cfg["L"], cfg["B"], cfg["E"], cfg["LC"]
    full = np.empty((L, B, E), np.float32)
    for cid in range(cfg["NC"]):
        o = np.asarray(results[cid]["out"]).reshape(LC, B, E)
        full[cid * LC:(cid + 1) * LC, :, :] = o
    return full


def get_built():
    if "full" not in _CACHE:
        cfg = make_cfg(FULL)
        _CACHE["full"] = (build_bass(cfg), cfg)
    return _CACHE["full"]


def kernel(**inputs):
    nc, cfg = get_built()
    in_maps = host_prep(cfg, inputs)
    res = run_bass_kernel_spmd(nc, in_maps, core_ids=list(range(cfg["NC"])))
    return assemble(cfg, res.results)

